# revision 1
# baseline (speedup 1.0000x reference)
"""MAB (multihead attention block) TRN2 kernel.

Sharding: 8 cores = batch (4) x query-half (2). Each core computes its
[1024, 256] output slice with zero cross-core communication (K/V
projections are recomputed by the 2 cores sharing a batch).

Layout strategy: everything transposed (features on partitions) so that
- projections contract d on partitions,
- scores come out as [k, q] (exp output directly usable as A@V rhs),
- softmax denominators via a ones-row appended to V (PE does the sum),
- LN stats via ones-vector matmuls + K=1 broadcast matmuls (PE),
- FFN contracts e on partitions directly.
All matmuls run in float32r (~1.3e-4 rel err, full PE rate).
"""

import numpy as np

import concourse.bass as bass
import concourse.mybir as mybir
import concourse.tile as tile
from concourse import bacc
from concourse.bass_utils import run_bass_kernel_spmd

F32 = mybir.dt.float32
F32R = mybir.dt.float32r
AF = mybir.ActivationFunctionType
ALU = mybir.AluOpType

B, NQ, NK, D = 4, 2048, 2048, 256
H, DH = 4, 64
S = NQ // 2          # queries per core
ET = D // 128        # feature tiles
QB = S // 512        # query blocks of 512
KT = NK // 128       # key tiles of 128
KB = NK // 512       # key blocks of 512
EPS = 1e-5
SCALE = 1.0 / np.sqrt(D)

_CACHE = {}


def _build(flags):
    (use_bq, use_bk, use_bv, use_bo, use_g0, use_g1) = flags
    nc = bacc.Bacc(None, target_bir_lowering=False)

    dQ = nc.dram_tensor("Qs", [S, D], F32, kind="ExternalInput")
    dK = nc.dram_tensor("Ks", [NK, D], F32, kind="ExternalInput")
    dW = {w: nc.dram_tensor(w, [D, D], F32, kind="ExternalInput")
          for w in ("Wq", "Wk", "Wv", "Wo")}
    dV = {v: nc.dram_tensor(v, [D], F32, kind="ExternalInput")
          for v in ("bq", "bk", "bv", "bo", "g0", "b0", "g1", "b1")}
    dO = nc.dram_tensor("Out", [S, D], F32, kind="ExternalOutput")

    with tile.TileContext(nc) as tc:
        with (
            tc.tile_pool(name="const", bufs=1) as cpool,
            tc.tile_pool(name="acts", bufs=1) as apool,
            tc.tile_pool(name="big", bufs=1) as bpool,
        ):
            # ---------------- constants / weights ----------------
            w_r = {}
            for w in ("Wq", "Wk", "Wv", "Wo"):
                w0 = cpool.tile([128, ET, D], F32)
                for dt in range(ET):
                    nc.sync.dma_start(w0[:, dt, :], dW[w].rearrange("e (dt d) -> dt d e", d=128)[dt])
                wr = cpool.tile([128, ET, D], F32R)
                nc.vector.tensor_copy(wr[:], w0[:])
                w_r[w] = wr
            vecs = {}
            for v in ("bq", "bk", "bv", "bo", "g0", "b0", "g1", "b1"):
                t = cpool.tile([128, ET], F32)
                nc.sync.dma_start(t[:], dV[v].rearrange("(et e) -> e et", e=128))
                vecs[v] = t
            ones1 = cpool.tile([1, 128], F32)
            nc.vector.memset(ones1[:], 1.0)
            ones1r = cpool.tile([1, 128], F32R)
            nc.vector.tensor_copy(ones1r[:], ones1[:])
            onesc = cpool.tile([128, 1], F32)
            nc.vector.memset(onesc[:], 1.0 / D)
            onescr = cpool.tile([128, 1], F32R)
            nc.vector.tensor_copy(onescr[:], onesc[:])
            onesf0 = cpool.tile([128, 128], F32)
            nc.vector.memset(onesf0[:], 1.0)
            onesF = cpool.tile([128, 128], F32R)
            nc.vector.tensor_copy(onesF[:], onesf0[:])
            onesFb = cpool.tile([128, 1], mybir.dt.bfloat16)
            nc.vector.tensor_copy(onesFb[:], onesf0[:, 0:1])
            epst = cpool.tile([1, 1], F32)
            nc.vector.memset(epst[:], EPS)

            # ---------------- activations: load + round ----------------
            QT = apool.tile([128, ET, S], F32R)
            KTr = apool.tile([128, ET, NK], F32R)
            with tc.tile_pool(name="stage", bufs=1) as stpool:
                qt0 = stpool.tile([128, ET, S], F32)
                for dt in range(ET):
                    nc.sync.dma_start(qt0[:, dt, :], dQ.rearrange("s (dt d) -> dt d s", d=128)[dt])
                nc.vector.tensor_copy(QT[:], qt0[:])
                kt0 = stpool.tile([128, ET, NK], F32)
                for dt in range(ET):
                    nc.sync.dma_start(kt0[:, dt, :], dK.rearrange("s (dt d) -> dt d s", d=128)[dt])
                nc.vector.tensor_copy(KTr[:], kt0[:])

            qT = bpool.tile([128, ET, S], F32R)       # projected q, transposed
            kT = bpool.tile([128, ET, NK], F32R)      # projected k, transposed
            v_sb = bpool.tile([128, KT, D], F32R)  # v natural [k, e]
            OT = bpool.tile([128, ET, S], F32R)       # attention out + residual
            O1 = bpool.tile([128, ET, S], F32R)       # LN0 out
            O2 = bpool.tile([128, ET, S], F32R)       # FFN+residual out
            O3 = bpool.tile([128, ET, S], F32)        # LN1 out (final)

            # ---------------- phase A: projections ----------------
            with tc.tile_pool(name="psA", bufs=4, space="PSUM") as psA:
                for et in range(ET):
                    for qb in range(QB):
                        ps = psA.tile([128, 512], F32)
                        for dt in range(ET):
                            nc.tensor.matmul(
                                ps[:], w_r["Wq"][:, dt, et * 128:(et + 1) * 128],
                                QT[:, dt, qb * 512:(qb + 1) * 512],
                                start=(dt == 0), stop=(dt == ET - 1))
                        dst = qT[:, et, qb * 512:(qb + 1) * 512]
                        if use_bq:
                            nc.vector.tensor_scalar_add(dst, ps[:], vecs["bq"][:, et:et + 1])
                        else:
                            nc.vector.tensor_copy(dst, ps[:])
                for et in range(ET):
                    for kb in range(KB):
                        ps = psA.tile([128, 512], F32)
                        for dt in range(ET):
                            nc.tensor.matmul(
                                ps[:], w_r["Wk"][:, dt, et * 128:(et + 1) * 128],
                                KTr[:, dt, kb * 512:(kb + 1) * 512],
                                start=(dt == 0), stop=(dt == ET - 1))
                        dst = kT[:, et, kb * 512:(kb + 1) * 512]
                        if use_bk:
                            nc.vector.tensor_scalar_add(dst, ps[:], vecs["bk"][:, et:et + 1])
                        else:
                            nc.vector.tensor_copy(dst, ps[:])
                for kt in range(KT):
                    ps = psA.tile([128, 512], F32)
                    for dt in range(ET):
                        nc.tensor.matmul(
                            ps[:, 0:256], KTr[:, dt, kt * 128:(kt + 1) * 128],
                            w_r["Wv"][:, dt, :],
                            start=(dt == 0), stop=(dt == ET - 1))
                    nc.vector.tensor_copy(v_sb[:, kt, :], ps[:, 0:256])

            # ---------------- phase B: attention ----------------
            with (
                tc.tile_pool(name="scps", bufs=1, space="PSUM") as scps,
                tc.tile_pool(name="accps", bufs=1, space="PSUM") as accps,
                tc.tile_pool(name="bcps", bufs=2, space="PSUM") as bcps,
                tc.tile_pool(name="ut", bufs=3) as utp,
                tc.tile_pool(name="sm", bufs=2) as smp,
            ):
                for hp in range(2):          # head pair = e-tile of kT/qT
                    for qb in range(QB):
                        qsl = slice(qb * 512, (qb + 1) * 512)
                        acc = [accps.tile([64, 512], F32, name=f"acc{_h}", tag=f"acc{_h}")
                               for _h in range(2)]
                        sms = [accps.tile([1, 512], F32, name=f"sms{_h}", tag=f"sms{_h}")
                               for _h in range(2)]
                        for kt in range(KT):
                            sc = scps.tile([128, 1024], F32)
                            for hh in range(2):
                                off = hh * 64
                                nc.tensor.matmul(
                                    sc[:, hh * 512:(hh + 1) * 512],
                                    kT[off:off + 64, hp, kt * 128:(kt + 1) * 128],
                                    qT[off:off + 64, hp, qsl],
                                    start=True, stop=True)
                            ut = utp.tile([128, 1024], F32R)
                            nc.scalar.activation(ut[:], sc[:], AF.Exp, scale=SCALE)
                            for hh in range(2):
                                h = hp * 2 + hh
                                nc.tensor.matmul(
                                    acc[hh][:],
                                    v_sb[:, kt, h * 64:(h + 1) * 64],
                                    ut[:, hh * 512:(hh + 1) * 512],
                                    start=(kt == 0), stop=(kt == KT - 1))
                                nc.tensor.matmul(
                                    sms[hh][:],
                                    onesF[:, 0:1],
                                    ut[:, hh * 512:(hh + 1) * 512],
                                    start=(kt == 0), stop=(kt == KT - 1))
                        for hh in range(2):
                            rec = smp.tile([1, 512], F32, name=f"rec{hh}", tag="rec")
                            nc.vector.reciprocal_approx_fast(out=rec[:], in_=sms[hh][:])
                            recr = smp.tile([1, 512], F32R, name=f"recr{hh}", tag="recr")
                            nc.vector.tensor_copy(recr[:], rec[:])
                            recB = bcps.tile([64, 512], F32, name=f"recB{hh}", tag="recB")
                            nc.tensor.matmul(recB[:], onesF[0:1, 0:64], recr[:],
                                             start=True, stop=True)
                            recS = smp.tile([64, 512], F32, name=f"recS{hh}", tag="recS")
                            nc.vector.tensor_copy(recS[:], recB[:])
                            tmp = smp.tile([64, 512], F32, name=f"tmp{hh}", tag="tmp")
                            nc.vector.tensor_mul(tmp[:], acc[hh][:], recS[:])
                            if hh == 0:
                                nc.vector.tensor_add(OT[0:64, hp, qsl], tmp[:],
                                                     qT[0:64, hp, qsl])
                            else:
                                tsh = smp.tile([128, 512], F32, name="tsh", tag="tsh")
                                nc.sync.dma_start(tsh[64:128, :], tmp[:])
                                nc.vector.tensor_add(OT[64:128, hp, qsl], tsh[64:128, :],
                                                     qT[64:128, hp, qsl])
                        if use_bv:
                            nc.vector.tensor_scalar_add(OT[:, hp, qsl], OT[:, hp, qsl],
                                                        vecs["bv"][:, hp:hp + 1])

            # ---------------- phase C: LN0 -> FFN -> LN1 ----------------
            def layernorm(x, y, gname, bname, use_g, out_f32):
                with (
                    tc.tile_pool(name="lnps", bufs=2, space="PSUM") as lnps,
                    tc.tile_pool(name="lnbc", bufs=2, space="PSUM") as lnbc,
                    tc.tile_pool(name="lnsm", bufs=2) as lnsm,
                    tc.tile_pool(name="lnsq", bufs=2) as lnsq,
                ):
                    for qb in range(QB):
                        qsl = slice(qb * 512, (qb + 1) * 512)
                        xsq = lnsq.tile([128, ET, 512], F32R)
                        for et in range(ET):
                            nc.vector.tensor_mul(xsq[:, et, :], x[:, et, qsl], x[:, et, qsl])
                        mus = lnps.tile([1, 512], F32)
                        sqs = lnps.tile([1, 512], F32)
                        for et in range(ET):
                            nc.tensor.matmul(mus[:], onescr[:], x[:, et, qsl],
                                             start=(et == 0), stop=(et == ET - 1))
                            nc.tensor.matmul(sqs[:], onescr[:], xsq[:, et, :],
                                             start=(et == 0), stop=(et == ET - 1))
                        mu = lnsm.tile([1, 512], F32)
                        nc.vector.tensor_copy(mu[:], mus[:])
                        musq = lnsm.tile([1, 512], F32)
                        nc.vector.tensor_mul(musq[:], mu[:], mu[:])
                        var = lnsm.tile([1, 512], F32)
                        nc.vector.tensor_sub(var[:], sqs[:], musq[:])
                        sd = lnsm.tile([1, 512], F32)
                        nc.scalar.activation(sd[:], var[:], AF.Sqrt, bias=epst[:])
                        rst = lnsm.tile([1, 512], F32)
                        nc.vector.reciprocal_approx_fast(out=rst[:], in_=sd[:])
                        mur = lnsm.tile([1, 512], F32R)
                        nc.vector.tensor_copy(mur[:], mu[:])
                        rstr = lnsm.tile([1, 512], F32R)
                        nc.vector.tensor_copy(rstr[:], rst[:])
                        muB = lnbc.tile([128, 512], F32)
                        nc.tensor.matmul(muB[:], ones1r[:], mur[:], start=True, stop=True)
                        rsB = lnbc.tile([128, 512], F32)
                        nc.tensor.matmul(rsB[:], ones1r[:], rstr[:], start=True, stop=True)
                        for et in range(ET):
                            cen = lnsm.tile([128, 512], F32)
                            nc.vector.tensor_sub(cen[:], x[:, et, qsl], muB[:])
                            dst = y[:, et, qsl]
                            nc.vector.tensor_mul(dst, cen[:], rsB[:])
                            if use_g:
                                nc.vector.tensor_scalar(
                                    dst, dst, vecs[gname][:, et:et + 1],
                                    vecs[bname][:, et:et + 1], ALU.mult, ALU.add)

            layernorm(OT, O1, "g0", "b0", use_g0, False)

            with (
                tc.tile_pool(name="ffps", bufs=2, space="PSUM") as ffps,
                tc.tile_pool(name="ffsm", bufs=2) as ffsm,
            ):
                for et in range(ET):
                    for qb in range(QB):
                        qsl = slice(qb * 512, (qb + 1) * 512)
                        ps = ffps.tile([128, 512], F32)
                        for dt in range(ET):
                            nc.tensor.matmul(
                                ps[:], w_r["Wo"][:, dt, et * 128:(et + 1) * 128],
                                O1[:, dt, qsl],
                                start=(dt == 0), stop=(dt == ET - 1))
                        ft = ffsm.tile([128, 512], F32)
                        nc.vector.tensor_scalar(
                            ft[:], ps[:], vecs["bo"][:, et:et + 1] if use_bo else 0.0,
                            0.0, ALU.add, ALU.max)
                        nc.vector.tensor_add(O2[:, et, qsl], O1[:, et, qsl], ft[:])

            layernorm(O2, O3, "g1", "b1", use_g1, True)

            for et in range(ET):
                nc.sync.dma_start(
                    dO.rearrange("s (et e) -> et e s", e=128)[et], O3[:, et, :])

    nc.compile()
    return nc


def kernel(Q, K, Wq, bq, Wk, bk, Wv, bv, Wo, bo, g0, b0, g1, b1):
    Q, K = np.asarray(Q), np.asarray(K)
    ws = {n: np.ascontiguousarray(np.asarray(v), dtype=np.float32)
          for n, v in (("Wq", Wq), ("Wk", Wk), ("Wv", Wv), ("Wo", Wo))}
    vs = {n: np.ascontiguousarray(np.asarray(v), dtype=np.float32)
          for n, v in (("bq", bq), ("bk", bk), ("bv", bv), ("bo", bo),
                       ("g0", g0), ("b0", b0), ("g1", g1), ("b1", b1))}
    flags = (bool(np.any(vs["bq"])), bool(np.any(vs["bk"])),
             bool(np.any(vs["bv"])), bool(np.any(vs["bo"])),
             bool(np.any(vs["g0"] != 1.0) or np.any(vs["b0"])),
             bool(np.any(vs["g1"] != 1.0) or np.any(vs["b1"])))
    if flags not in _CACHE:
        _CACHE[flags] = _build(flags)
    nc = _CACHE[flags]

    in_maps = []
    for b in range(B):
        for half in range(2):
            m = {"Qs": np.ascontiguousarray(Q[b, half * S:(half + 1) * S], dtype=np.float32),
                 "Ks": np.ascontiguousarray(K[b], dtype=np.float32)}
            m.update(ws)
            m.update(vs)
            in_maps.append(m)

    res = run_bass_kernel_spmd(nc, in_maps, list(range(8)))
    out = np.empty((B, NQ, D), dtype=np.float32)
    for i in range(8):
        b, half = divmod(i, 2)
        out[b, half * S:(half + 1) * S] = res.results[i]["Out"]
    return out



# revision 8
# speedup vs baseline: 3.1222x; 3.1222x over previous
"""MAB (multihead attention block) TRN2 kernel.

Sharding: 8 cores = batch (4) x query-half (2). Each core computes its
[1024, 256] output slice with zero cross-core communication (K/V
projections are recomputed by the 2 cores sharing a batch).

Layout strategy: everything transposed (features on partitions) so that
- projections contract d on partitions,
- scores come out as [k, q] (exp output directly usable as A@V rhs),
- softmax denominators via a ones-row appended to V (PE does the sum),
- LN stats via ones-vector matmuls + K=1 broadcast matmuls (PE),
- FFN contracts e on partitions directly.
All matmuls run in float32r (~1.3e-4 rel err, full PE rate).
"""

import numpy as np

import concourse.bass as bass
import concourse.mybir as mybir
import concourse.tile as tile
from concourse import bacc
from concourse import masks
from concourse.bass_utils import run_bass_kernel_spmd

F32 = mybir.dt.float32
F32R = mybir.dt.float32r
AF = mybir.ActivationFunctionType
ALU = mybir.AluOpType

B, NQ, NK, D = 4, 2048, 2048, 256
H, DH = 4, 64
S = NQ // 2          # queries per core
ET = D // 128        # feature tiles
QB = S // 512        # query blocks of 512
KT = NK // 128       # key tiles of 128
KB = NK // 512       # key blocks of 512
EPS = 1e-5
SCALE = 1.0 / np.sqrt(D)

_CACHE = {}


def _build(flags):
    (use_bq, use_bk, use_bv, use_bo, use_g0, use_g1) = flags
    nc = bacc.Bacc(None, target_bir_lowering=False)

    dQ = nc.dram_tensor("Qs", [S, D], F32, kind="ExternalInput")
    dK = nc.dram_tensor("Ks", [NK, D], F32, kind="ExternalInput")
    dW = {w: nc.dram_tensor(w, [D, D], F32, kind="ExternalInput")
          for w in ("Wq", "Wk", "Wv", "Wo")}
    dV = {v: nc.dram_tensor(v, [D], F32, kind="ExternalInput")
          for v in ("bq", "bk", "bv", "bo", "g0", "b0", "g1", "b1")}
    dO = nc.dram_tensor("Out", [S, D], F32, kind="ExternalOutput")

    with tile.TileContext(nc) as tc:
        with (
            tc.tile_pool(name="const", bufs=1) as cpool,
            tc.tile_pool(name="acts", bufs=1) as apool,
            tc.tile_pool(name="big", bufs=1) as bpool,
        ):
            # ---------------- constants / weights ----------------
            id128 = cpool.tile([128, 128], F32)
            masks.make_identity(nc, id128[:])
            w_r = {}
            for w in ("Wq", "Wk", "Wv", "Wo"):
                wr = cpool.tile([128, ET, D], F32R, name=f"wr_{w}", tag=f"wr_{w}")
                w_r[w] = wr
            with (
                tc.tile_pool(name="wstage", bufs=1) as wstpool,
                tc.tile_pool(name="wtps", bufs=4, space="PSUM") as wtps,
            ):
                for w in ("Wq", "Wk", "Wv", "Wo"):
                    wn = wstpool.tile([128, ET, D], F32, name=f"wn_{w}", tag=f"wn_{w}")
                    nc.sync.dma_start(
                        wn[:], dW[w].rearrange("(et p) d -> p et d", p=128))
                    for dt in range(ET):
                        ps = wtps.tile([128, 512], F32)
                        for et in range(ET):
                            nc.tensor.transpose(
                                ps[:, et * 128:(et + 1) * 128],
                                wn[:, et, dt * 128:(dt + 1) * 128], id128[:])
                        nc.vector.tensor_copy(w_r[w][:, dt, :], ps[:, 0:D])
            vecs = {}
            for v in ("bq", "bk", "bv", "bo", "g0", "b0", "g1", "b1"):
                t = cpool.tile([128, ET], F32)
                nc.sync.dma_start(t[:], dV[v].rearrange("(et e) -> e et", e=128))
                vecs[v] = t
            ones1 = cpool.tile([1, 128], F32)
            nc.vector.memset(ones1[:], 1.0)
            ones1r = cpool.tile([1, 128], F32R)
            nc.vector.tensor_copy(ones1r[:], ones1[:])
            onesc = cpool.tile([128, 1], F32)
            nc.vector.memset(onesc[:], 1.0 / D)
            onescr = cpool.tile([128, 1], F32R)
            nc.vector.tensor_copy(onescr[:], onesc[:])
            onesf0 = cpool.tile([128, 128], F32)
            nc.vector.memset(onesf0[:], 1.0)
            onesF = cpool.tile([128, 128], F32R)
            nc.vector.tensor_copy(onesF[:], onesf0[:])
            onesFb = cpool.tile([128, 1], mybir.dt.bfloat16)
            nc.vector.tensor_copy(onesFb[:], onesf0[:, 0:1])
            epst = cpool.tile([1, 1], F32)
            nc.vector.memset(epst[:], EPS)

            # ---------------- activations: load + round ----------------
            QT = apool.tile([128, ET, S], F32R)
            KTr = apool.tile([128, ET, NK], F32R)
            with (
                tc.tile_pool(name="stage", bufs=1) as stpool,
                tc.tile_pool(name="tps", bufs=4, space="PSUM") as tps,
            ):
                qn = stpool.tile([128, 8, D], F32)
                for g in range(2):
                    nc.sync.dma_start(
                        qn[:, g * 4:(g + 1) * 4, :],
                        dQ.rearrange("(g st p) d -> g p st d", p=128, st=4)[g])
                kn = stpool.tile([128, KT, D], F32)
                for g in range(4):
                    nc.sync.dma_start(
                        kn[:, g * 4:(g + 1) * 4, :],
                        dK.rearrange("(g st p) d -> g p st d", p=128, st=4)[g])
                for dt in range(ET):
                    for g in range(2):
                        ps = tps.tile([128, 512], F32)
                        for j in range(4):
                            nc.tensor.transpose(
                                ps[:, j * 128:(j + 1) * 128],
                                qn[:, g * 4 + j, dt * 128:(dt + 1) * 128], id128[:])
                        nc.vector.tensor_copy(
                            QT[:, dt, g * 512:(g + 1) * 512], ps[:])
                for dt in range(ET):
                    for g in range(4):
                        ps = tps.tile([128, 512], F32)
                        for j in range(4):
                            nc.tensor.transpose(
                                ps[:, j * 128:(j + 1) * 128],
                                kn[:, g * 4 + j, dt * 128:(dt + 1) * 128], id128[:])
                        nc.vector.tensor_copy(
                            KTr[:, dt, g * 512:(g + 1) * 512], ps[:])

            qT = bpool.tile([128, ET, S], F32R)       # projected q, transposed
            kT = bpool.tile([128, ET, NK], F32R)      # projected k, transposed
            v_sb = bpool.tile([128, KT, D], F32R)  # v natural [k, e]
            OT = bpool.tile([128, ET, S], F32R)       # attention out + residual
            O1 = bpool.tile([128, ET, S], F32R)       # LN0 out
            O2 = bpool.tile([128, ET, S], F32R)       # FFN+residual out
            O3 = bpool.tile([128, ET, S], F32)        # LN1 out (final)

            # ---------------- phase A: projections ----------------
            with tc.tile_pool(name="psA", bufs=4, space="PSUM") as psA:
                for et in range(ET):
                    for qb in range(QB):
                        ps = psA.tile([128, 512], F32)
                        for dt in range(ET):
                            nc.tensor.matmul(
                                ps[:], w_r["Wq"][:, dt, et * 128:(et + 1) * 128],
                                QT[:, dt, qb * 512:(qb + 1) * 512],
                                start=(dt == 0), stop=(dt == ET - 1))
                        dst = qT[:, et, qb * 512:(qb + 1) * 512]
                        if use_bq:
                            nc.vector.tensor_scalar_add(dst, ps[:], vecs["bq"][:, et:et + 1])
                        else:
                            nc.vector.tensor_copy(dst, ps[:])
                for et in range(ET):
                    for kb in range(KB):
                        ps = psA.tile([128, 512], F32)
                        for dt in range(ET):
                            nc.tensor.matmul(
                                ps[:], w_r["Wk"][:, dt, et * 128:(et + 1) * 128],
                                KTr[:, dt, kb * 512:(kb + 1) * 512],
                                start=(dt == 0), stop=(dt == ET - 1))
                        dst = kT[:, et, kb * 512:(kb + 1) * 512]
                        if use_bk:
                            nc.vector.tensor_scalar_add(dst, ps[:], vecs["bk"][:, et:et + 1])
                        else:
                            nc.vector.tensor_copy(dst, ps[:])
                for kt in range(KT):
                    ps = psA.tile([128, 512], F32)
                    for dt in range(ET):
                        nc.tensor.matmul(
                            ps[:, 0:256], KTr[:, dt, kt * 128:(kt + 1) * 128],
                            w_r["Wv"][:, dt, :],
                            start=(dt == 0), stop=(dt == ET - 1))
                    nc.vector.tensor_copy(v_sb[:, kt, :], ps[:, 0:256])

            # ---------------- phase B: attention ----------------
            with (
                tc.tile_pool(name="scps", bufs=1, space="PSUM") as scps,
                tc.tile_pool(name="accps", bufs=1, space="PSUM") as accps,
                tc.tile_pool(name="bcps", bufs=2, space="PSUM") as bcps,
                tc.tile_pool(name="ut", bufs=3) as utp,
                tc.tile_pool(name="sm", bufs=2) as smp,
            ):
                for hp in range(2):          # head pair = e-tile of kT/qT
                    for qb in range(QB):
                        qsl = slice(qb * 512, (qb + 1) * 512)
                        acc = [accps.tile([64, 512], F32, name=f"acc{_h}", tag=f"acc{_h}")
                               for _h in range(2)]
                        sms = [accps.tile([1, 512], F32, name=f"sms{_h}", tag=f"sms{_h}")
                               for _h in range(2)]
                        for kt in range(KT):
                            sc = scps.tile([128, 1024], F32)
                            for hh in range(2):
                                off = hh * 64
                                nc.tensor.matmul(
                                    sc[:, hh * 512:(hh + 1) * 512],
                                    kT[off:off + 64, hp, kt * 128:(kt + 1) * 128],
                                    qT[off:off + 64, hp, qsl],
                                    start=True, stop=True)
                            ut = utp.tile([128, 1024], F32R)
                            nc.scalar.activation(ut[:], sc[:], AF.Exp, scale=SCALE)
                            for hh in range(2):
                                h = hp * 2 + hh
                                nc.tensor.matmul(
                                    acc[hh][:],
                                    v_sb[:, kt, h * 64:(h + 1) * 64],
                                    ut[:, hh * 512:(hh + 1) * 512],
                                    start=(kt == 0), stop=(kt == KT - 1))
                                nc.tensor.matmul(
                                    sms[hh][:],
                                    onesF[:, 0:1],
                                    ut[:, hh * 512:(hh + 1) * 512],
                                    start=(kt == 0), stop=(kt == KT - 1))
                        for hh in range(2):
                            rec = smp.tile([1, 512], F32, name=f"rec{hh}", tag="rec")
                            nc.vector.reciprocal_approx_fast(out=rec[:], in_=sms[hh][:])
                            recr = smp.tile([1, 512], F32R, name=f"recr{hh}", tag="recr")
                            nc.vector.tensor_copy(recr[:], rec[:])
                            recB = bcps.tile([64, 512], F32, name=f"recB{hh}", tag="recB")
                            nc.tensor.matmul(recB[:], onesF[0:1, 0:64], recr[:],
                                             start=True, stop=True)
                            recS = smp.tile([64, 512], F32, name=f"recS{hh}", tag="recS")
                            nc.vector.tensor_copy(recS[:], recB[:])
                            tmp = smp.tile([64, 512], F32, name=f"tmp{hh}", tag="tmp")
                            nc.vector.tensor_mul(tmp[:], acc[hh][:], recS[:])
                            if hh == 0:
                                nc.vector.tensor_add(OT[0:64, hp, qsl], tmp[:],
                                                     qT[0:64, hp, qsl])
                            else:
                                tsh = smp.tile([128, 512], F32, name="tsh", tag="tsh")
                                nc.sync.dma_start(tsh[64:128, :], tmp[:])
                                nc.vector.tensor_add(OT[64:128, hp, qsl], tsh[64:128, :],
                                                     qT[64:128, hp, qsl])
                        if use_bv:
                            nc.vector.tensor_scalar_add(OT[:, hp, qsl], OT[:, hp, qsl],
                                                        vecs["bv"][:, hp:hp + 1])

            # ---------------- phase C: LN0 -> FFN -> LN1 ----------------
            def layernorm(x, y, gname, bname, use_g, out_f32):
                with (
                    tc.tile_pool(name="lnps", bufs=2, space="PSUM") as lnps,
                    tc.tile_pool(name="lnbc", bufs=2, space="PSUM") as lnbc,
                    tc.tile_pool(name="lnsm", bufs=2) as lnsm,
                    tc.tile_pool(name="lnsq", bufs=2) as lnsq,
                ):
                    for qb in range(QB):
                        qsl = slice(qb * 512, (qb + 1) * 512)
                        xsq = lnsq.tile([128, ET, 512], F32R)
                        for et in range(ET):
                            nc.vector.tensor_mul(xsq[:, et, :], x[:, et, qsl], x[:, et, qsl])
                        mus = lnps.tile([1, 512], F32)
                        sqs = lnps.tile([1, 512], F32)
                        for et in range(ET):
                            nc.tensor.matmul(mus[:], onescr[:], x[:, et, qsl],
                                             start=(et == 0), stop=(et == ET - 1))
                            nc.tensor.matmul(sqs[:], onescr[:], xsq[:, et, :],
                                             start=(et == 0), stop=(et == ET - 1))
                        mu = lnsm.tile([1, 512], F32)
                        nc.vector.tensor_copy(mu[:], mus[:])
                        musq = lnsm.tile([1, 512], F32)
                        nc.vector.tensor_mul(musq[:], mu[:], mu[:])
                        var = lnsm.tile([1, 512], F32)
                        nc.vector.tensor_sub(var[:], sqs[:], musq[:])
                        sd = lnsm.tile([1, 512], F32)
                        nc.scalar.activation(sd[:], var[:], AF.Sqrt, bias=epst[:])
                        rst = lnsm.tile([1, 512], F32)
                        nc.vector.reciprocal_approx_fast(out=rst[:], in_=sd[:])
                        mur = lnsm.tile([1, 512], F32R)
                        nc.vector.tensor_copy(mur[:], mu[:])
                        rstr = lnsm.tile([1, 512], F32R)
                        nc.vector.tensor_copy(rstr[:], rst[:])
                        muB = lnbc.tile([128, 512], F32)
                        nc.tensor.matmul(muB[:], ones1r[:], mur[:], start=True, stop=True)
                        rsB = lnbc.tile([128, 512], F32)
                        nc.tensor.matmul(rsB[:], ones1r[:], rstr[:], start=True, stop=True)
                        for et in range(ET):
                            cen = lnsm.tile([128, 512], F32)
                            nc.vector.tensor_sub(cen[:], x[:, et, qsl], muB[:])
                            dst = y[:, et, qsl]
                            nc.vector.tensor_mul(dst, cen[:], rsB[:])
                            if use_g:
                                nc.vector.tensor_scalar(
                                    dst, dst, vecs[gname][:, et:et + 1],
                                    vecs[bname][:, et:et + 1], ALU.mult, ALU.add)

            layernorm(OT, O1, "g0", "b0", use_g0, False)

            with (
                tc.tile_pool(name="ffps", bufs=2, space="PSUM") as ffps,
                tc.tile_pool(name="ffsm", bufs=2) as ffsm,
            ):
                for et in range(ET):
                    for qb in range(QB):
                        qsl = slice(qb * 512, (qb + 1) * 512)
                        ps = ffps.tile([128, 512], F32)
                        for dt in range(ET):
                            nc.tensor.matmul(
                                ps[:], w_r["Wo"][:, dt, et * 128:(et + 1) * 128],
                                O1[:, dt, qsl],
                                start=(dt == 0), stop=(dt == ET - 1))
                        ft = ffsm.tile([128, 512], F32)
                        nc.vector.tensor_scalar(
                            ft[:], ps[:], vecs["bo"][:, et:et + 1] if use_bo else 0.0,
                            0.0, ALU.add, ALU.max)
                        nc.vector.tensor_add(O2[:, et, qsl], O1[:, et, qsl], ft[:])

            layernorm(O2, O3, "g1", "b1", use_g1, True)

            On = bpool.tile([128, 8, D], F32)
            with tc.tile_pool(name="ops", bufs=2, space="PSUM") as ops:
                for st in range(8):
                    po = ops.tile([128, 256], F32)
                    for dt in range(ET):
                        nc.tensor.transpose(
                            po[:, dt * 128:(dt + 1) * 128],
                            O3[:, dt, st * 128:(st + 1) * 128], id128[:])
                    nc.vector.tensor_copy(On[:, st, :], po[:])
            for g in range(2):
                nc.sync.dma_start(
                    dO.rearrange("(g st p) d -> g p st d", p=128, st=4)[g],
                    On[:, g * 4:(g + 1) * 4, :])

    nc.compile()
    return nc


def kernel(Q, K, Wq, bq, Wk, bk, Wv, bv, Wo, bo, g0, b0, g1, b1):
    Q, K = np.asarray(Q), np.asarray(K)
    ws = {n: np.ascontiguousarray(np.asarray(v), dtype=np.float32)
          for n, v in (("Wq", Wq), ("Wk", Wk), ("Wv", Wv), ("Wo", Wo))}
    vs = {n: np.ascontiguousarray(np.asarray(v), dtype=np.float32)
          for n, v in (("bq", bq), ("bk", bk), ("bv", bv), ("bo", bo),
                       ("g0", g0), ("b0", b0), ("g1", g1), ("b1", b1))}
    flags = (bool(np.any(vs["bq"])), bool(np.any(vs["bk"])),
             bool(np.any(vs["bv"])), bool(np.any(vs["bo"])),
             bool(np.any(vs["g0"] != 1.0) or np.any(vs["b0"])),
             bool(np.any(vs["g1"] != 1.0) or np.any(vs["b1"])))
    if flags not in _CACHE:
        _CACHE[flags] = _build(flags)
    nc = _CACHE[flags]

    in_maps = []
    for b in range(B):
        for half in range(2):
            m = {"Qs": np.ascontiguousarray(Q[b, half * S:(half + 1) * S], dtype=np.float32),
                 "Ks": np.ascontiguousarray(K[b], dtype=np.float32)}
            m.update(ws)
            m.update(vs)
            in_maps.append(m)

    res = run_bass_kernel_spmd(nc, in_maps, list(range(8)))
    out = np.empty((B, NQ, D), dtype=np.float32)
    for i in range(8):
        b, half = divmod(i, 2)
        out[b, half * S:(half + 1) * S] = res.results[i]["Out"]
    return out



# revision 10
# speedup vs baseline: 3.3744x; 1.0808x over previous
"""MAB (multihead attention block) TRN2 kernel.

Sharding: 8 cores = batch (4) x query-half (2). Each core computes its
[1024, 256] output slice with zero cross-core communication (K/V
projections are recomputed by the 2 cores sharing a batch).

Layout strategy: everything transposed (features on partitions) so that
- projections contract d on partitions,
- scores come out as [k, q] (exp output directly usable as A@V rhs),
- softmax denominators via a ones-row appended to V (PE does the sum),
- LN stats via ones-vector matmuls + K=1 broadcast matmuls (PE),
- FFN contracts e on partitions directly.
All matmuls run in float32r (~1.3e-4 rel err, full PE rate).
"""

import numpy as np

import concourse.bass as bass
import concourse.mybir as mybir
import concourse.tile as tile
from concourse import bacc
from concourse import masks
from concourse.bass_utils import run_bass_kernel_spmd

F32 = mybir.dt.float32
F32R = mybir.dt.float32r
AF = mybir.ActivationFunctionType
ALU = mybir.AluOpType

B, NQ, NK, D = 4, 2048, 2048, 256
H, DH = 4, 64
S = NQ // 2          # queries per core
ET = D // 128        # feature tiles
QB = S // 512        # query blocks of 512
KT = NK // 128       # key tiles of 128
KB = NK // 512       # key blocks of 512
EPS = 1e-5
SCALE = 1.0 / np.sqrt(D)

_CACHE = {}


def _build(flags):
    (use_bq, use_bk, use_bv, use_bo, use_g0, use_g1) = flags
    nc = bacc.Bacc(None, target_bir_lowering=False)

    dQ = nc.dram_tensor("Qs", [S, D], F32, kind="ExternalInput")
    dK = nc.dram_tensor("Ks", [NK, D], F32, kind="ExternalInput")
    dW = {w: nc.dram_tensor(w, [D, D], F32, kind="ExternalInput")
          for w in ("Wq", "Wk", "Wv", "Wo")}
    dV = {v: nc.dram_tensor(v, [D], F32, kind="ExternalInput")
          for v in ("bq", "bk", "bv", "bo", "g0", "b0", "g1", "b1")}
    dO = nc.dram_tensor("Out", [S, D], F32, kind="ExternalOutput")

    with tile.TileContext(nc) as tc:
        with (
            tc.tile_pool(name="const", bufs=1) as cpool,
            tc.tile_pool(name="acts", bufs=1) as apool,
            tc.tile_pool(name="big", bufs=1) as bpool,
        ):
            # ---------------- constants / weights ----------------
            id128 = cpool.tile([128, 128], F32)
            masks.make_identity(nc, id128[:])
            w_r = {}
            for w in ("Wq", "Wk", "Wv", "Wo"):
                wr = cpool.tile([128, ET, D], F32R, name=f"wr_{w}", tag=f"wr_{w}")
                w_r[w] = wr
            with (
                tc.tile_pool(name="wstage", bufs=1) as wstpool,
                tc.tile_pool(name="wtps", bufs=4, space="PSUM") as wtps,
            ):
                for w in ("Wq", "Wk", "Wv", "Wo"):
                    wn = wstpool.tile([128, ET, D], F32, name=f"wn_{w}", tag=f"wn_{w}")
                    nc.sync.dma_start(
                        wn[:], dW[w].rearrange("(et p) d -> p et d", p=128))
                    for dt in range(ET):
                        ps = wtps.tile([128, 512], F32)
                        for et in range(ET):
                            nc.tensor.transpose(
                                ps[:, et * 128:(et + 1) * 128],
                                wn[:, et, dt * 128:(dt + 1) * 128], id128[:])
                        nc.vector.tensor_copy(w_r[w][:, dt, :], ps[:, 0:D])
            vecs = {}
            for v in ("bq", "bk", "bv", "bo", "g0", "b0", "g1", "b1"):
                t = cpool.tile([128, ET], F32)
                nc.sync.dma_start(t[:], dV[v].rearrange("(et e) -> e et", e=128))
                vecs[v] = t
            ones1 = cpool.tile([1, 128], F32)
            nc.vector.memset(ones1[:], 1.0)
            ones1r = cpool.tile([1, 128], F32R)
            nc.vector.tensor_copy(ones1r[:], ones1[:])
            onesc = cpool.tile([128, 1], F32)
            nc.vector.memset(onesc[:], 1.0 / D)
            onescr = cpool.tile([128, 1], F32R)
            nc.vector.tensor_copy(onescr[:], onesc[:])
            onesf0 = cpool.tile([128, 128], F32)
            nc.vector.memset(onesf0[:], 1.0)
            onesF = cpool.tile([128, 128], F32R)
            nc.vector.tensor_copy(onesF[:], onesf0[:])
            onesFb = cpool.tile([128, 1], mybir.dt.bfloat16)
            nc.vector.tensor_copy(onesFb[:], onesf0[:, 0:1])
            epst = cpool.tile([1, 1], F32)
            nc.vector.memset(epst[:], EPS)

            # ---------------- activations: load + round ----------------
            QT = apool.tile([128, ET, S], F32R)
            KTr = apool.tile([128, ET, NK], F32R)
            with (
                tc.tile_pool(name="stage", bufs=1) as stpool,
                tc.tile_pool(name="tps", bufs=4, space="PSUM") as tps,
            ):
                qn = stpool.tile([128, 8, D], F32)
                for g in range(2):
                    nc.sync.dma_start(
                        qn[:, g * 4:(g + 1) * 4, :],
                        dQ.rearrange("(g st p) d -> g p st d", p=128, st=4)[g])
                kn = stpool.tile([128, KT, D], F32)
                for g in range(4):
                    nc.sync.dma_start(
                        kn[:, g * 4:(g + 1) * 4, :],
                        dK.rearrange("(g st p) d -> g p st d", p=128, st=4)[g])
                for dt in range(ET):
                    for g in range(2):
                        ps = tps.tile([128, 512], F32)
                        for j in range(4):
                            nc.tensor.transpose(
                                ps[:, j * 128:(j + 1) * 128],
                                qn[:, g * 4 + j, dt * 128:(dt + 1) * 128], id128[:])
                        nc.vector.tensor_copy(
                            QT[:, dt, g * 512:(g + 1) * 512], ps[:])
                for dt in range(ET):
                    for g in range(4):
                        ps = tps.tile([128, 512], F32)
                        for j in range(4):
                            nc.tensor.transpose(
                                ps[:, j * 128:(j + 1) * 128],
                                kn[:, g * 4 + j, dt * 128:(dt + 1) * 128], id128[:])
                        nc.vector.tensor_copy(
                            KTr[:, dt, g * 512:(g + 1) * 512], ps[:])

            qT = bpool.tile([128, ET, S], F32R)       # projected q, transposed
            kT = bpool.tile([128, ET, NK], F32R)      # projected k, transposed
            v_sb = bpool.tile([128, KT, H, 65], F32R)  # v [k, h, dh + ones col]
            OT = bpool.tile([128, ET, S], F32R)       # attention out + residual
            O1 = bpool.tile([128, ET, S], F32R)       # LN0 out
            O2 = bpool.tile([128, ET, S], F32R)       # FFN+residual out
            O3 = bpool.tile([128, ET, S], F32)        # LN1 out (final)
            nc.vector.tensor_copy(v_sb[:, :, :, 64:65], onesf0[:, 0:KT * H])

            # ---------------- phase A: projections ----------------
            with tc.tile_pool(name="psA", bufs=4, space="PSUM") as psA:
                for et in range(ET):
                    for qb in range(QB):
                        ps = psA.tile([128, 512], F32)
                        for dt in range(ET):
                            nc.tensor.matmul(
                                ps[:], w_r["Wq"][:, dt, et * 128:(et + 1) * 128],
                                QT[:, dt, qb * 512:(qb + 1) * 512],
                                start=(dt == 0), stop=(dt == ET - 1))
                        dst = qT[:, et, qb * 512:(qb + 1) * 512]
                        if use_bq:
                            nc.vector.tensor_scalar_add(dst, ps[:], vecs["bq"][:, et:et + 1])
                        else:
                            nc.vector.tensor_copy(dst, ps[:])
                for et in range(ET):
                    for kb in range(KB):
                        ps = psA.tile([128, 512], F32)
                        for dt in range(ET):
                            nc.tensor.matmul(
                                ps[:], w_r["Wk"][:, dt, et * 128:(et + 1) * 128],
                                KTr[:, dt, kb * 512:(kb + 1) * 512],
                                start=(dt == 0), stop=(dt == ET - 1))
                        dst = kT[:, et, kb * 512:(kb + 1) * 512]
                        if use_bk:
                            nc.vector.tensor_scalar_add(dst, ps[:], vecs["bk"][:, et:et + 1])
                        else:
                            nc.vector.tensor_copy(dst, ps[:])
                for kt in range(KT):
                    ps = psA.tile([128, 512], F32)
                    for dt in range(ET):
                        nc.tensor.matmul(
                            ps[:, 0:256], KTr[:, dt, kt * 128:(kt + 1) * 128],
                            w_r["Wv"][:, dt, :],
                            start=(dt == 0), stop=(dt == ET - 1))
                    nc.vector.tensor_copy(v_sb[:, kt, :, 0:64], ps[:, 0:256])

            # ---------------- phase B: attention ----------------
            with (
                tc.tile_pool(name="scps", bufs=1, space="PSUM") as scps,
                tc.tile_pool(name="accps", bufs=1, space="PSUM") as accps,
                tc.tile_pool(name="bcps", bufs=2, space="PSUM") as bcps,
                tc.tile_pool(name="ut", bufs=3) as utp,
                tc.tile_pool(name="sm", bufs=2) as smp,
            ):
                for hp in range(2):          # head pair = e-tile of kT/qT
                    for qb in range(QB):
                        qsl = slice(qb * 512, (qb + 1) * 512)
                        acc = [accps.tile([65, 512], F32, name=f"acc{_h}", tag=f"acc{_h}")
                               for _h in range(2)]
                        for kt in range(KT):
                            sc = scps.tile([128, 1024], F32)
                            for hh in range(2):
                                off = hh * 64
                                nc.tensor.matmul(
                                    sc[:, hh * 512:(hh + 1) * 512],
                                    kT[off:off + 64, hp, kt * 128:(kt + 1) * 128],
                                    qT[off:off + 64, hp, qsl],
                                    start=True, stop=True)
                            ut = utp.tile([128, 1024], F32R)
                            nc.scalar.activation(ut[:], sc[:], AF.Exp, scale=SCALE)
                            for hh in range(2):
                                h = hp * 2 + hh
                                nc.tensor.matmul(
                                    acc[hh][:],
                                    v_sb[:, kt, h, :],
                                    ut[:, hh * 512:(hh + 1) * 512],
                                    start=(kt == 0), stop=(kt == KT - 1))
                        for hh in range(2):
                            dcp = smp.tile([65, 512], F32, name=f"dcp{hh}", tag="dcp")
                            nc.vector.tensor_copy(dcp[64:65, :], acc[hh][64:65, :])
                            den0 = smp.tile([1, 512], F32, name=f"den0{hh}", tag="den0")
                            nc.sync.dma_start(den0[0:1, :], dcp[64:65, :])
                            rec = smp.tile([1, 512], F32, name=f"rec{hh}", tag="rec")
                            nc.vector.reciprocal_approx_fast(out=rec[:], in_=den0[:])
                            recr = smp.tile([1, 512], F32R, name=f"recr{hh}", tag="recr")
                            nc.vector.tensor_copy(recr[:], rec[:])
                            recB = bcps.tile([64, 512], F32, name=f"recB{hh}", tag="recB")
                            nc.tensor.matmul(recB[:], onesF[0:1, 0:64], recr[:],
                                             start=True, stop=True)
                            recS = smp.tile([64, 512], F32, name=f"recS{hh}", tag="recS")
                            nc.vector.tensor_copy(recS[:], recB[:])
                            tmp = smp.tile([64, 512], F32, name=f"tmp{hh}", tag="tmp")
                            nc.vector.tensor_mul(tmp[:], acc[hh][0:64, :], recS[:])
                            if hh == 0:
                                nc.vector.tensor_add(OT[0:64, hp, qsl], tmp[:],
                                                     qT[0:64, hp, qsl])
                            else:
                                tsh = smp.tile([128, 512], F32, name="tsh", tag="tsh")
                                nc.sync.dma_start(tsh[64:128, :], tmp[:])
                                nc.vector.tensor_add(OT[64:128, hp, qsl], tsh[64:128, :],
                                                     qT[64:128, hp, qsl])
                        if use_bv:
                            nc.vector.tensor_scalar_add(OT[:, hp, qsl], OT[:, hp, qsl],
                                                        vecs["bv"][:, hp:hp + 1])

            # ---------------- phase C: LN0 -> FFN -> LN1 ----------------
            def layernorm(x, y, gname, bname, use_g, out_f32):
                with (
                    tc.tile_pool(name="lnps", bufs=2, space="PSUM") as lnps,
                    tc.tile_pool(name="lnbc", bufs=2, space="PSUM") as lnbc,
                    tc.tile_pool(name="lnsm", bufs=2) as lnsm,
                    tc.tile_pool(name="lnsq", bufs=2) as lnsq,
                ):
                    for qb in range(QB):
                        qsl = slice(qb * 512, (qb + 1) * 512)
                        xsq = lnsq.tile([128, ET, 512], F32R)
                        for et in range(ET):
                            nc.vector.tensor_mul(xsq[:, et, :], x[:, et, qsl], x[:, et, qsl])
                        mus = lnps.tile([1, 512], F32)
                        sqs = lnps.tile([1, 512], F32)
                        for et in range(ET):
                            nc.tensor.matmul(mus[:], onescr[:], x[:, et, qsl],
                                             start=(et == 0), stop=(et == ET - 1))
                            nc.tensor.matmul(sqs[:], onescr[:], xsq[:, et, :],
                                             start=(et == 0), stop=(et == ET - 1))
                        mu = lnsm.tile([1, 512], F32)
                        nc.vector.tensor_copy(mu[:], mus[:])
                        musq = lnsm.tile([1, 512], F32)
                        nc.vector.tensor_mul(musq[:], mu[:], mu[:])
                        var = lnsm.tile([1, 512], F32)
                        nc.vector.tensor_sub(var[:], sqs[:], musq[:])
                        sd = lnsm.tile([1, 512], F32)
                        nc.scalar.activation(sd[:], var[:], AF.Sqrt, bias=epst[:])
                        rst = lnsm.tile([1, 512], F32)
                        nc.vector.reciprocal_approx_fast(out=rst[:], in_=sd[:])
                        mur = lnsm.tile([1, 512], F32R)
                        nc.vector.tensor_copy(mur[:], mu[:])
                        rstr = lnsm.tile([1, 512], F32R)
                        nc.vector.tensor_copy(rstr[:], rst[:])
                        muB = lnbc.tile([128, 512], F32)
                        nc.tensor.matmul(muB[:], ones1r[:], mur[:], start=True, stop=True)
                        rsB = lnbc.tile([128, 512], F32)
                        nc.tensor.matmul(rsB[:], ones1r[:], rstr[:], start=True, stop=True)
                        for et in range(ET):
                            cen = lnsm.tile([128, 512], F32)
                            nc.vector.tensor_sub(cen[:], x[:, et, qsl], muB[:])
                            dst = y[:, et, qsl]
                            nc.vector.tensor_mul(dst, cen[:], rsB[:])
                            if use_g:
                                nc.vector.tensor_scalar(
                                    dst, dst, vecs[gname][:, et:et + 1],
                                    vecs[bname][:, et:et + 1], ALU.mult, ALU.add)

            layernorm(OT, O1, "g0", "b0", use_g0, False)

            with (
                tc.tile_pool(name="ffps", bufs=2, space="PSUM") as ffps,
                tc.tile_pool(name="ffsm", bufs=2) as ffsm,
            ):
                for et in range(ET):
                    for qb in range(QB):
                        qsl = slice(qb * 512, (qb + 1) * 512)
                        ps = ffps.tile([128, 512], F32)
                        for dt in range(ET):
                            nc.tensor.matmul(
                                ps[:], w_r["Wo"][:, dt, et * 128:(et + 1) * 128],
                                O1[:, dt, qsl],
                                start=(dt == 0), stop=(dt == ET - 1))
                        ft = ffsm.tile([128, 512], F32)
                        nc.vector.tensor_scalar(
                            ft[:], ps[:], vecs["bo"][:, et:et + 1] if use_bo else 0.0,
                            0.0, ALU.add, ALU.max)
                        nc.vector.tensor_add(O2[:, et, qsl], O1[:, et, qsl], ft[:])

            layernorm(O2, O3, "g1", "b1", use_g1, True)

            On = bpool.tile([128, 8, D], F32)
            with tc.tile_pool(name="ops", bufs=2, space="PSUM") as ops:
                for st in range(8):
                    po = ops.tile([128, 256], F32)
                    for dt in range(ET):
                        nc.tensor.transpose(
                            po[:, dt * 128:(dt + 1) * 128],
                            O3[:, dt, st * 128:(st + 1) * 128], id128[:])
                    nc.vector.tensor_copy(On[:, st, :], po[:])
            for g in range(2):
                nc.sync.dma_start(
                    dO.rearrange("(g st p) d -> g p st d", p=128, st=4)[g],
                    On[:, g * 4:(g + 1) * 4, :])

    nc.compile()
    return nc


def kernel(Q, K, Wq, bq, Wk, bk, Wv, bv, Wo, bo, g0, b0, g1, b1):
    Q, K = np.asarray(Q), np.asarray(K)
    ws = {n: np.ascontiguousarray(np.asarray(v), dtype=np.float32)
          for n, v in (("Wq", Wq), ("Wk", Wk), ("Wv", Wv), ("Wo", Wo))}
    vs = {n: np.ascontiguousarray(np.asarray(v), dtype=np.float32)
          for n, v in (("bq", bq), ("bk", bk), ("bv", bv), ("bo", bo),
                       ("g0", g0), ("b0", b0), ("g1", g1), ("b1", b1))}
    flags = (bool(np.any(vs["bq"])), bool(np.any(vs["bk"])),
             bool(np.any(vs["bv"])), bool(np.any(vs["bo"])),
             bool(np.any(vs["g0"] != 1.0) or np.any(vs["b0"])),
             bool(np.any(vs["g1"] != 1.0) or np.any(vs["b1"])))
    if flags not in _CACHE:
        _CACHE[flags] = _build(flags)
    nc = _CACHE[flags]

    in_maps = []
    for b in range(B):
        for half in range(2):
            m = {"Qs": np.ascontiguousarray(Q[b, half * S:(half + 1) * S], dtype=np.float32),
                 "Ks": np.ascontiguousarray(K[b], dtype=np.float32)}
            m.update(ws)
            m.update(vs)
            in_maps.append(m)

    res = run_bass_kernel_spmd(nc, in_maps, list(range(8)))
    out = np.empty((B, NQ, D), dtype=np.float32)
    for i in range(8):
        b, half = divmod(i, 2)
        out[b, half * S:(half + 1) * S] = res.results[i]["Out"]
    return out



# revision 12
# speedup vs baseline: 4.2309x; 1.2538x over previous
"""MAB (multihead attention block) TRN2 kernel.

Sharding: 8 cores = batch (4) x query-half (2). Each core computes its
[1024, 256] output slice with zero cross-core communication (K/V
projections are recomputed by the 2 cores sharing a batch).

Layout strategy: everything transposed (features on partitions) so that
- projections contract d on partitions,
- scores come out as [k, q] (exp output directly usable as A@V rhs),
- softmax denominators via a ones-row appended to V (PE does the sum),
- LN stats via ones-vector matmuls + K=1 broadcast matmuls (PE),
- FFN contracts e on partitions directly.
All matmuls run in float32r (~1.3e-4 rel err, full PE rate).
"""

import numpy as np

import concourse.bass as bass
import concourse.mybir as mybir
import concourse.tile as tile
from concourse import bacc
from concourse import masks
from concourse.bass_utils import run_bass_kernel_spmd

F32 = mybir.dt.float32
F32R = mybir.dt.float32r
AF = mybir.ActivationFunctionType
ALU = mybir.AluOpType

B, NQ, NK, D = 4, 2048, 2048, 256
H, DH = 4, 64
S = NQ // 2          # queries per core
ET = D // 128        # feature tiles
QB = S // 512        # query blocks of 512
KT = NK // 128       # key tiles of 128
KB = NK // 512       # key blocks of 512
EPS = 1e-5
SCALE = 1.0 / np.sqrt(D)

_CACHE = {}


def _build(flags):
    (use_bq, use_bk, use_bv, use_bo, use_g0, use_g1) = flags
    nc = bacc.Bacc(None, target_bir_lowering=False)

    dQ = nc.dram_tensor("Qs", [S, D], F32, kind="ExternalInput")
    dK = nc.dram_tensor("Ks", [NK, D], F32, kind="ExternalInput")
    dW = {w: nc.dram_tensor(w, [D, D], F32, kind="ExternalInput")
          for w in ("Wq", "Wk", "Wv", "Wo")}
    dV = {v: nc.dram_tensor(v, [D], F32, kind="ExternalInput")
          for v in ("bq", "bk", "bv", "bo", "g0", "b0", "g1", "b1")}
    dO = nc.dram_tensor("Out", [S, D], F32, kind="ExternalOutput")

    with tile.TileContext(nc) as tc:
        with (
            tc.tile_pool(name="const", bufs=1) as cpool,
            tc.tile_pool(name="acts", bufs=1) as apool,
            tc.tile_pool(name="big", bufs=1) as bpool,
        ):
            # ---------------- constants / weights ----------------
            id128 = cpool.tile([128, 128], F32)
            masks.make_identity(nc, id128[:])
            w_r = {}
            for w in ("Wq", "Wk", "Wv", "Wo"):
                wr = cpool.tile([128, ET, D], F32R, name=f"wr_{w}", tag=f"wr_{w}")
                w_r[w] = wr
            with (
                tc.tile_pool(name="wstage", bufs=1) as wstpool,
                tc.tile_pool(name="wtps", bufs=4, space="PSUM") as wtps,
            ):
                for w in ("Wq", "Wk", "Wv", "Wo"):
                    wn = wstpool.tile([128, ET, D], F32, name=f"wn_{w}", tag=f"wn_{w}")
                    nc.sync.dma_start(
                        wn[:], dW[w].rearrange("(et p) d -> p et d", p=128))
                    for dt in range(ET):
                        ps = wtps.tile([128, 512], F32)
                        for et in range(ET):
                            nc.tensor.transpose(
                                ps[:, et * 128:(et + 1) * 128],
                                wn[:, et, dt * 128:(dt + 1) * 128], id128[:])
                        nc.any.tensor_copy(w_r[w][:, dt, :], ps[:, 0:D])
            vecs = {}
            for v in ("bq", "bk", "bv", "bo", "g0", "b0", "g1", "b1"):
                t = cpool.tile([128, ET], F32)
                nc.sync.dma_start(t[:], dV[v].rearrange("(et e) -> e et", e=128))
                vecs[v] = t
            ones1 = cpool.tile([1, 128], F32)
            nc.vector.memset(ones1[:], 1.0)
            ones1r = cpool.tile([1, 128], F32R)
            nc.vector.tensor_copy(ones1r[:], ones1[:])
            onesc = cpool.tile([128, 1], F32)
            nc.vector.memset(onesc[:], 1.0 / D)
            onescr = cpool.tile([128, 1], F32R)
            nc.vector.tensor_copy(onescr[:], onesc[:])
            onesf0 = cpool.tile([128, 128], F32)
            nc.vector.memset(onesf0[:], 1.0)
            onesF = cpool.tile([128, 128], F32R)
            nc.vector.tensor_copy(onesF[:], onesf0[:])
            onesFb = cpool.tile([128, 1], mybir.dt.bfloat16)
            nc.vector.tensor_copy(onesFb[:], onesf0[:, 0:1])
            epst = cpool.tile([1, 1], F32)
            nc.vector.memset(epst[:], EPS)

            # ---------------- activations: load + round ----------------
            QT = apool.tile([128, ET, S], F32R)
            KTr = apool.tile([128, ET, NK], F32R)
            with (
                tc.tile_pool(name="stage", bufs=1) as stpool,
                tc.tile_pool(name="tps", bufs=4, space="PSUM") as tps,
            ):
                qn = stpool.tile([128, 8, D], F32)
                for g in range(2):
                    nc.sync.dma_start(
                        qn[:, g * 4:(g + 1) * 4, :],
                        dQ.rearrange("(g st p) d -> g p st d", p=128, st=4)[g])
                kn = stpool.tile([128, KT, D], F32)
                for g in range(4):
                    nc.sync.dma_start(
                        kn[:, g * 4:(g + 1) * 4, :],
                        dK.rearrange("(g st p) d -> g p st d", p=128, st=4)[g])
                for dt in range(ET):
                    for g in range(2):
                        ps = tps.tile([128, 512], F32)
                        for j in range(4):
                            nc.tensor.transpose(
                                ps[:, j * 128:(j + 1) * 128],
                                qn[:, g * 4 + j, dt * 128:(dt + 1) * 128], id128[:])
                        nc.any.tensor_copy(
                            QT[:, dt, g * 512:(g + 1) * 512], ps[:])
                for dt in range(ET):
                    for g in range(4):
                        ps = tps.tile([128, 512], F32)
                        for j in range(4):
                            nc.tensor.transpose(
                                ps[:, j * 128:(j + 1) * 128],
                                kn[:, g * 4 + j, dt * 128:(dt + 1) * 128], id128[:])
                        nc.any.tensor_copy(
                            KTr[:, dt, g * 512:(g + 1) * 512], ps[:])

            qT = bpool.tile([128, ET, S], F32R)       # projected q, transposed
            kT = bpool.tile([128, ET, NK], F32R)      # projected k, transposed
            v_sb = bpool.tile([128, KT, H, 65], F32R)  # v [k, h, dh + ones col]
            OT = bpool.tile([128, ET, S], F32R)       # attention out + residual
            O1 = bpool.tile([128, ET, S], F32R)       # LN0 out
            O2 = bpool.tile([128, ET, S], F32R)       # FFN+residual out
            O3 = bpool.tile([128, ET, S], F32)        # LN1 out (final)
            nc.vector.tensor_copy(v_sb[:, :, :, 64:65], onesf0[:, 0:KT * H])

            # ---------------- phase A: projections ----------------
            with tc.tile_pool(name="psA", bufs=4, space="PSUM") as psA:
                for et in range(ET):
                    for qb in range(QB):
                        ps = psA.tile([128, 512], F32)
                        for dt in range(ET):
                            nc.tensor.matmul(
                                ps[:], w_r["Wq"][:, dt, et * 128:(et + 1) * 128],
                                QT[:, dt, qb * 512:(qb + 1) * 512],
                                start=(dt == 0), stop=(dt == ET - 1))
                        dst = qT[:, et, qb * 512:(qb + 1) * 512]
                        if use_bq:
                            nc.any.tensor_scalar_add(dst, ps[:], vecs["bq"][:, et:et + 1])
                        else:
                            nc.any.tensor_copy(dst, ps[:])
                for et in range(ET):
                    for kb in range(KB):
                        ps = psA.tile([128, 512], F32)
                        for dt in range(ET):
                            nc.tensor.matmul(
                                ps[:], w_r["Wk"][:, dt, et * 128:(et + 1) * 128],
                                KTr[:, dt, kb * 512:(kb + 1) * 512],
                                start=(dt == 0), stop=(dt == ET - 1))
                        dst = kT[:, et, kb * 512:(kb + 1) * 512]
                        if use_bk:
                            nc.any.tensor_scalar_add(dst, ps[:], vecs["bk"][:, et:et + 1])
                        else:
                            nc.any.tensor_copy(dst, ps[:])
                for kt in range(KT):
                    ps = psA.tile([128, 512], F32)
                    for dt in range(ET):
                        nc.tensor.matmul(
                            ps[:, 0:256], KTr[:, dt, kt * 128:(kt + 1) * 128],
                            w_r["Wv"][:, dt, :],
                            start=(dt == 0), stop=(dt == ET - 1))
                    nc.any.tensor_copy(v_sb[:, kt, :, 0:64], ps[:, 0:256])

            # ------------- phase B+C fused: per 512-query block -------------
            # PSUM budget (8 banks): sc 2x2 + acc 2 + shared "c" pool 2.
            On = bpool.tile([128, 8, D], F32)
            with (
                tc.tile_pool(name="scps", bufs=2, space="PSUM") as scps,
                tc.tile_pool(name="accps", bufs=1, space="PSUM") as accps,
                tc.tile_pool(name="cps", bufs=2, space="PSUM") as cps,
                tc.tile_pool(name="ut", bufs=3) as utp,
                tc.tile_pool(name="sm", bufs=2) as smp,
                tc.tile_pool(name="lnsm", bufs=2) as lnsm,
                tc.tile_pool(name="lnsq", bufs=2) as lnsq,
                tc.tile_pool(name="ffsm", bufs=2) as ffsm,
            ):
                def attention(hp, qb):
                    qsl = slice(qb * 512, (qb + 1) * 512)
                    acc = [accps.tile([65, 512], F32, name=f"acc{_h}", tag=f"acc{_h}")
                           for _h in range(2)]
                    uts = {}
                    # software-pipelined: scores/exp for kt overlap A@V for kt-1
                    for kt in range(KT + 1):
                        if kt < KT:
                            sc = scps.tile([128, 1024], F32, name="sc", tag="sc")
                            for hh in range(2):
                                off = hh * 64
                                nc.tensor.matmul(
                                    sc[:, hh * 512:(hh + 1) * 512],
                                    kT[off:off + 64, hp, kt * 128:(kt + 1) * 128],
                                    qT[off:off + 64, hp, qsl],
                                    start=True, stop=True)
                            ut = utp.tile([128, 1024], F32R)
                            nc.scalar.activation(ut[:], sc[:], AF.Exp, scale=SCALE)
                            uts[kt] = ut
                        if kt >= 1:
                            utp_ = uts.pop(kt - 1)
                            for hh in range(2):
                                h = hp * 2 + hh
                                nc.tensor.matmul(
                                    acc[hh][:],
                                    v_sb[:, kt - 1, h, :],
                                    utp_[:, hh * 512:(hh + 1) * 512],
                                    start=(kt - 1 == 0), stop=(kt - 1 == KT - 1))
                    for hh in range(2):
                        dcp = smp.tile([65, 512], F32, name=f"dcp{hh}", tag="dcp")
                        nc.vector.tensor_copy(dcp[64:65, :], acc[hh][64:65, :])
                        den0 = smp.tile([1, 512], F32, name=f"den0{hh}", tag="den0")
                        nc.sync.dma_start(den0[0:1, :], dcp[64:65, :])
                        lnd = smp.tile([1, 512], F32, name=f"lnd{hh}", tag="lnd")
                        nc.scalar.activation(lnd[:], den0[:], AF.Ln)
                        recr = smp.tile([1, 512], F32R, name=f"recr{hh}", tag="recr")
                        nc.scalar.activation(recr[:], lnd[:], AF.Exp, scale=-1.0)
                        recB = cps.tile([64, 512], F32, name=f"recB{hh}", tag="c")
                        nc.tensor.matmul(recB[:], onesF[0:1, 0:64], recr[:],
                                         start=True, stop=True)
                        recS = smp.tile([64, 512], F32, name=f"recS{hh}", tag="recS")
                        nc.any.tensor_copy(recS[:], recB[:])
                        tmp = smp.tile([64, 512], F32, name=f"tmp{hh}", tag="tmp")
                        nc.any.tensor_mul(tmp[:], acc[hh][0:64, :], recS[:])
                        if hh == 0:
                            nc.any.tensor_add(OT[0:64, hp, qsl], tmp[:],
                                              qT[0:64, hp, qsl])
                        else:
                            tsh = smp.tile([128, 512], F32, name="tsh", tag="tsh")
                            nc.sync.dma_start(tsh[64:128, :], tmp[:])
                            nc.any.tensor_add(OT[64:128, hp, qsl], tsh[64:128, :],
                                              qT[64:128, hp, qsl])
                    if use_bv:
                        nc.vector.tensor_scalar_add(OT[:, hp, qsl], OT[:, hp, qsl],
                                                    vecs["bv"][:, hp:hp + 1])

                def layernorm(x, y, qb, gname, bname, use_g):
                    qsl = slice(qb * 512, (qb + 1) * 512)
                    xsq = lnsq.tile([128, ET, 512], F32R)
                    for et in range(ET):
                        nc.any.tensor_mul(xsq[:, et, :], x[:, et, qsl], x[:, et, qsl])
                    mus = cps.tile([1, 512], F32, name="mus", tag="c")
                    sqs = cps.tile([1, 512], F32, name="sqs", tag="c")
                    for et in range(ET):
                        nc.tensor.matmul(mus[:], onescr[:], x[:, et, qsl],
                                         start=(et == 0), stop=(et == ET - 1))
                        nc.tensor.matmul(sqs[:], onescr[:], xsq[:, et, :],
                                         start=(et == 0), stop=(et == ET - 1))
                    musq = lnsm.tile([1, 512], F32, name="musq", tag="musq")
                    nc.scalar.activation(musq[:], mus[:], AF.Square)
                    mur = lnsm.tile([1, 512], F32R, name="mur", tag="mur")
                    nc.any.tensor_copy(mur[:], mus[:])
                    var = lnsm.tile([1, 512], F32, name="var", tag="var")
                    nc.vector.tensor_sub(var[:], sqs[:], musq[:])
                    lnv = lnsm.tile([1, 512], F32, name="lnv", tag="lnv")
                    nc.scalar.activation(lnv[:], var[:], AF.Ln, bias=epst[:])
                    rstr = lnsm.tile([1, 512], F32R, name="rstr", tag="rstr")
                    nc.scalar.activation(rstr[:], lnv[:], AF.Exp, scale=-0.5)
                    muB = cps.tile([128, 512], F32, name="muB", tag="c")
                    nc.tensor.matmul(muB[:], ones1r[:], mur[:], start=True, stop=True)
                    rsB = cps.tile([128, 512], F32, name="rsB", tag="c")
                    nc.tensor.matmul(rsB[:], ones1r[:], rstr[:], start=True, stop=True)
                    for et in range(ET):
                        cen = lnsm.tile([128, 512], F32, name="cen", tag="cen")
                        nc.any.tensor_sub(cen[:], x[:, et, qsl], muB[:])
                        dst = y[:, et, qsl]
                        nc.any.tensor_mul(dst, cen[:], rsB[:])
                        if use_g:
                            nc.vector.tensor_scalar(
                                dst, dst, vecs[gname][:, et:et + 1],
                                vecs[bname][:, et:et + 1], ALU.mult, ALU.add)

                for qb in range(QB):
                    qsl = slice(qb * 512, (qb + 1) * 512)
                    for hp in range(2):
                        attention(hp, qb)
                    layernorm(OT, O1, qb, "g0", "b0", use_g0)
                    for et in range(ET):
                        ps = cps.tile([128, 512], F32, name="ffps", tag="c")
                        for dt in range(ET):
                            nc.tensor.matmul(
                                ps[:], w_r["Wo"][:, dt, et * 128:(et + 1) * 128],
                                O1[:, dt, qsl],
                                start=(dt == 0), stop=(dt == ET - 1))
                        ft = ffsm.tile([128, 512], F32, name="ft", tag="ft")
                        nc.vector.tensor_scalar(
                            ft[:], ps[:], vecs["bo"][:, et:et + 1] if use_bo else 0.0,
                            0.0, ALU.add, ALU.max)
                        nc.any.tensor_add(O2[:, et, qsl], O1[:, et, qsl], ft[:])
                    layernorm(O2, O3, qb, "g1", "b1", use_g1)
                    for st in range(4):
                        po = cps.tile([128, 256], F32, name="po", tag="c")
                        for dt in range(ET):
                            nc.tensor.transpose(
                                po[:, dt * 128:(dt + 1) * 128],
                                O3[:, dt, qb * 512 + st * 128:qb * 512 + (st + 1) * 128],
                                id128[:])
                        nc.any.tensor_copy(On[:, qb * 4 + st, :], po[:])
                    nc.sync.dma_start(
                        dO.rearrange("(g st p) d -> g p st d", p=128, st=4)[qb],
                        On[:, qb * 4:(qb + 1) * 4, :])

    nc.compile()
    return nc


def kernel(Q, K, Wq, bq, Wk, bk, Wv, bv, Wo, bo, g0, b0, g1, b1):
    Q, K = np.asarray(Q), np.asarray(K)
    ws = {n: np.ascontiguousarray(np.asarray(v), dtype=np.float32)
          for n, v in (("Wq", Wq), ("Wk", Wk), ("Wv", Wv), ("Wo", Wo))}
    vs = {n: np.ascontiguousarray(np.asarray(v), dtype=np.float32)
          for n, v in (("bq", bq), ("bk", bk), ("bv", bv), ("bo", bo),
                       ("g0", g0), ("b0", b0), ("g1", g1), ("b1", b1))}
    flags = (bool(np.any(vs["bq"])), bool(np.any(vs["bk"])),
             bool(np.any(vs["bv"])), bool(np.any(vs["bo"])),
             bool(np.any(vs["g0"] != 1.0) or np.any(vs["b0"])),
             bool(np.any(vs["g1"] != 1.0) or np.any(vs["b1"])))
    if flags not in _CACHE:
        _CACHE[flags] = _build(flags)
    nc = _CACHE[flags]

    in_maps = []
    for b in range(B):
        for half in range(2):
            m = {"Qs": np.ascontiguousarray(Q[b, half * S:(half + 1) * S], dtype=np.float32),
                 "Ks": np.ascontiguousarray(K[b], dtype=np.float32)}
            m.update(ws)
            m.update(vs)
            in_maps.append(m)

    res = run_bass_kernel_spmd(nc, in_maps, list(range(8)))
    out = np.empty((B, NQ, D), dtype=np.float32)
    for i in range(8):
        b, half = divmod(i, 2)
        out[b, half * S:(half + 1) * S] = res.results[i]["Out"]
    return out



# revision 13
# speedup vs baseline: 4.6464x; 1.0982x over previous
"""MAB (multihead attention block) TRN2 kernel.

Sharding: 8 cores = batch (4) x query-half (2). Each core computes its
[1024, 256] output slice with zero cross-core communication (K/V
projections are recomputed by the 2 cores sharing a batch).

Layout strategy: everything transposed (features on partitions) so that
- projections contract d on partitions,
- scores come out as [k, q] (exp output directly usable as A@V rhs),
- softmax denominators via a ones-row appended to V (PE does the sum),
- LN stats via ones-vector matmuls + K=1 broadcast matmuls (PE),
- FFN contracts e on partitions directly.
All matmuls run in float32r (~1.3e-4 rel err, full PE rate).
"""

import numpy as np

import concourse.bass as bass
import concourse.mybir as mybir
import concourse.tile as tile
from concourse import bacc
from concourse import masks
from concourse.bass_utils import run_bass_kernel_spmd

F32 = mybir.dt.float32
F32R = mybir.dt.float32r
AF = mybir.ActivationFunctionType
ALU = mybir.AluOpType

B, NQ, NK, D = 4, 2048, 2048, 256
H, DH = 4, 64
S = NQ // 2          # queries per core
ET = D // 128        # feature tiles
QB = S // 512        # query blocks of 512
KT = NK // 128       # key tiles of 128
KB = NK // 512       # key blocks of 512
EPS = 1e-5
SCALE = 1.0 / np.sqrt(D)

_CACHE = {}


def _build(flags):
    (use_bq, use_bk, use_bv, use_bo, use_g0, use_g1) = flags
    nc = bacc.Bacc(None, target_bir_lowering=False)

    dQ = nc.dram_tensor("Qs", [S, D], F32, kind="ExternalInput")
    dK = nc.dram_tensor("Ks", [NK, D], F32, kind="ExternalInput")
    dW = {w: nc.dram_tensor(w, [D, D], F32, kind="ExternalInput")
          for w in ("Wq", "Wk", "Wv", "Wo")}
    dV = {v: nc.dram_tensor(v, [D], F32, kind="ExternalInput")
          for v in ("bq", "bk", "bv", "bo", "g0", "b0", "g1", "b1")}
    dO = nc.dram_tensor("Out", [S, D], F32, kind="ExternalOutput")

    with tile.TileContext(nc) as tc:
        with (
            tc.tile_pool(name="const", bufs=1) as cpool,
            tc.tile_pool(name="acts", bufs=1) as apool,
            tc.tile_pool(name="big", bufs=1) as bpool,
        ):
            # ---------------- constants / weights ----------------
            id128 = cpool.tile([128, 128], F32)
            masks.make_identity(nc, id128[:])
            w_r = {}
            for w in ("Wq", "Wk", "Wv", "Wo"):
                wr = cpool.tile([128, ET, D], F32R, name=f"wr_{w}", tag=f"wr_{w}")
                w_r[w] = wr
            with (
                tc.tile_pool(name="wstage", bufs=1) as wstpool,
                tc.tile_pool(name="wtps", bufs=4, space="PSUM") as wtps,
            ):
                for w in ("Wq", "Wk", "Wv", "Wo"):
                    wn = wstpool.tile([128, ET, D], F32, name=f"wn_{w}", tag=f"wn_{w}")
                    nc.sync.dma_start(
                        wn[:], dW[w].rearrange("(et p) d -> p et d", p=128))
                    for dt in range(ET):
                        ps = wtps.tile([128, 512], F32)
                        for et in range(ET):
                            nc.tensor.transpose(
                                ps[:, et * 128:(et + 1) * 128],
                                wn[:, et, dt * 128:(dt + 1) * 128], id128[:])
                        if dt == 0:
                            nc.vector.tensor_copy(w_r[w][:, dt, :], ps[:, 0:D])
                        else:
                            nc.scalar.copy(w_r[w][:, dt, :], ps[:, 0:D])
            vecs = {}
            for v in ("bq", "bk", "bv", "bo", "g0", "b0", "g1", "b1"):
                t = cpool.tile([128, ET], F32)
                nc.sync.dma_start(t[:], dV[v].rearrange("(et e) -> e et", e=128))
                vecs[v] = t
            ones1 = cpool.tile([1, 128], F32)
            nc.vector.memset(ones1[:], 1.0)
            ones1r = cpool.tile([1, 128], F32R)
            nc.vector.tensor_copy(ones1r[:], ones1[:])
            onesc = cpool.tile([128, 1], F32)
            nc.vector.memset(onesc[:], 1.0 / D)
            onescr = cpool.tile([128, 1], F32R)
            nc.vector.tensor_copy(onescr[:], onesc[:])
            onesf0 = cpool.tile([128, 128], F32)
            nc.vector.memset(onesf0[:], 1.0)
            onesF = cpool.tile([128, 128], F32R)
            nc.vector.tensor_copy(onesF[:], onesf0[:])
            onesFb = cpool.tile([128, 1], mybir.dt.bfloat16)
            nc.vector.tensor_copy(onesFb[:], onesf0[:, 0:1])
            epst = cpool.tile([1, 1], F32)
            nc.vector.memset(epst[:], EPS)

            # ---------------- activations: load + round ----------------
            QT = apool.tile([128, ET, S], F32R)
            KTr = apool.tile([128, ET, NK], F32R)
            with (
                tc.tile_pool(name="stage", bufs=1) as stpool,
                tc.tile_pool(name="tps", bufs=4, space="PSUM") as tps,
            ):
                qn = stpool.tile([128, 8, D], F32)
                for g in range(2):
                    nc.sync.dma_start(
                        qn[:, g * 4:(g + 1) * 4, :],
                        dQ.rearrange("(g st p) d -> g p st d", p=128, st=4)[g])
                kn = stpool.tile([128, KT, D], F32)
                for g in range(4):
                    nc.sync.dma_start(
                        kn[:, g * 4:(g + 1) * 4, :],
                        dK.rearrange("(g st p) d -> g p st d", p=128, st=4)[g])
                for dt in range(ET):
                    for g in range(2):
                        ps = tps.tile([128, 512], F32)
                        for j in range(4):
                            nc.tensor.transpose(
                                ps[:, j * 128:(j + 1) * 128],
                                qn[:, g * 4 + j, dt * 128:(dt + 1) * 128], id128[:])
                        (nc.vector.tensor_copy if g % 2 == 0 else nc.scalar.copy)(
                            QT[:, dt, g * 512:(g + 1) * 512], ps[:])
                for dt in range(ET):
                    for g in range(4):
                        ps = tps.tile([128, 512], F32)
                        for j in range(4):
                            nc.tensor.transpose(
                                ps[:, j * 128:(j + 1) * 128],
                                kn[:, g * 4 + j, dt * 128:(dt + 1) * 128], id128[:])
                        (nc.vector.tensor_copy if g % 2 == 0 else nc.scalar.copy)(
                            KTr[:, dt, g * 512:(g + 1) * 512], ps[:])

            qT = bpool.tile([128, ET, S], F32R)       # projected q, transposed
            kT = bpool.tile([128, ET, NK], F32R)      # projected k, transposed
            v_sb = bpool.tile([128, KT, H, 65], F32R)  # v [k, h, dh + ones col]
            OT = bpool.tile([128, ET, S], F32R)       # attention out + residual
            O1 = bpool.tile([128, ET, S], F32R)       # LN0 out
            O2 = bpool.tile([128, ET, S], F32R)       # FFN+residual out
            O3 = bpool.tile([128, ET, S], F32)        # LN1 out (final)
            nc.vector.tensor_copy(v_sb[:, :, :, 64:65], onesf0[:, 0:KT * H])

            # ---------------- phase A: projections ----------------
            with tc.tile_pool(name="psA", bufs=4, space="PSUM") as psA:
                for et in range(ET):
                    for qb in range(QB):
                        ps = psA.tile([128, 512], F32)
                        for dt in range(ET):
                            nc.tensor.matmul(
                                ps[:], w_r["Wq"][:, dt, et * 128:(et + 1) * 128],
                                QT[:, dt, qb * 512:(qb + 1) * 512],
                                start=(dt == 0), stop=(dt == ET - 1))
                        dst = qT[:, et, qb * 512:(qb + 1) * 512]
                        if use_bq:
                            nc.vector.tensor_scalar_add(dst, ps[:], vecs["bq"][:, et:et + 1])
                        else:
                            (nc.vector.tensor_copy if qb % 2 == 0 else nc.scalar.copy)(
                                dst, ps[:])
                for et in range(ET):
                    for kb in range(KB):
                        ps = psA.tile([128, 512], F32)
                        for dt in range(ET):
                            nc.tensor.matmul(
                                ps[:], w_r["Wk"][:, dt, et * 128:(et + 1) * 128],
                                KTr[:, dt, kb * 512:(kb + 1) * 512],
                                start=(dt == 0), stop=(dt == ET - 1))
                        dst = kT[:, et, kb * 512:(kb + 1) * 512]
                        if use_bk:
                            nc.vector.tensor_scalar_add(dst, ps[:], vecs["bk"][:, et:et + 1])
                        else:
                            (nc.vector.tensor_copy if kb % 2 == 0 else nc.scalar.copy)(
                                dst, ps[:])
                for kt in range(KT):
                    ps = psA.tile([128, 512], F32)
                    for dt in range(ET):
                        nc.tensor.matmul(
                            ps[:, 0:256], KTr[:, dt, kt * 128:(kt + 1) * 128],
                            w_r["Wv"][:, dt, :],
                            start=(dt == 0), stop=(dt == ET - 1))
                    (nc.vector.tensor_copy if kt % 2 == 0 else nc.scalar.copy)(
                        v_sb[:, kt, :, 0:64], ps[:, 0:256])

            # ------------- phase B+C fused: per 512-query block -------------
            # PSUM budget (8 banks): sc 2x2 + acc 2 + shared "c" pool 2.
            On = bpool.tile([128, 8, D], F32)
            with (
                tc.tile_pool(name="scps", bufs=2, space="PSUM") as scps,
                tc.tile_pool(name="accps", bufs=1, space="PSUM") as accps,
                tc.tile_pool(name="cps", bufs=2, space="PSUM") as cps,
                tc.tile_pool(name="ut", bufs=3) as utp,
                tc.tile_pool(name="sm", bufs=2) as smp,
                tc.tile_pool(name="lnsm", bufs=2) as lnsm,
                tc.tile_pool(name="lnsq", bufs=2) as lnsq,
                tc.tile_pool(name="ffsm", bufs=2) as ffsm,
            ):
                def attention(hp, qb):
                    qsl = slice(qb * 512, (qb + 1) * 512)
                    acc = [accps.tile([65, 512], F32, name=f"acc{_h}", tag=f"acc{_h}")
                           for _h in range(2)]
                    uts = {}
                    # software-pipelined: scores/exp for kt overlap A@V for kt-1
                    for kt in range(KT + 1):
                        if kt < KT:
                            sc = scps.tile([128, 1024], F32, name="sc", tag="sc")
                            for hh in range(2):
                                off = hh * 64
                                nc.tensor.matmul(
                                    sc[:, hh * 512:(hh + 1) * 512],
                                    kT[off:off + 64, hp, kt * 128:(kt + 1) * 128],
                                    qT[off:off + 64, hp, qsl],
                                    start=True, stop=True)
                            ut = utp.tile([128, 1024], F32R)
                            nc.scalar.activation(ut[:], sc[:], AF.Exp, scale=SCALE)
                            uts[kt] = ut
                        if kt >= 1:
                            utp_ = uts.pop(kt - 1)
                            for hh in range(2):
                                h = hp * 2 + hh
                                nc.tensor.matmul(
                                    acc[hh][:],
                                    v_sb[:, kt - 1, h, :],
                                    utp_[:, hh * 512:(hh + 1) * 512],
                                    start=(kt - 1 == 0), stop=(kt - 1 == KT - 1))
                    for hh in range(2):
                        dcp = smp.tile([65, 512], F32, name=f"dcp{hh}", tag="dcp")
                        nc.vector.tensor_copy(dcp[64:65, :], acc[hh][64:65, :])
                        den0 = smp.tile([1, 512], F32, name=f"den0{hh}", tag="den0")
                        nc.sync.dma_start(den0[0:1, :], dcp[64:65, :])
                        rec = smp.tile([1, 512], F32, name=f"rec{hh}", tag="rec")
                        nc.vector.reciprocal_approx_fast(out=rec[:], in_=den0[:])
                        recr = smp.tile([1, 512], F32R, name=f"recr{hh}", tag="recr")
                        nc.vector.tensor_copy(recr[:], rec[:])
                        recB = cps.tile([64, 512], F32, name=f"recB{hh}", tag="c")
                        nc.tensor.matmul(recB[:], onesF[0:1, 0:64], recr[:],
                                         start=True, stop=True)
                        recS = smp.tile([64, 512], F32, name=f"recS{hh}", tag="recS")
                        nc.vector.tensor_copy(recS[:], recB[:])
                        tmp = smp.tile([64, 512], F32, name=f"tmp{hh}", tag="tmp")
                        nc.vector.tensor_mul(tmp[:], acc[hh][0:64, :], recS[:])
                        if hh == 0:
                            nc.gpsimd.tensor_add(OT[0:64, hp, qsl], tmp[:],
                                                 qT[0:64, hp, qsl])
                        else:
                            tsh = smp.tile([128, 512], F32, name="tsh", tag="tsh")
                            nc.sync.dma_start(tsh[64:128, :], tmp[:])
                            nc.gpsimd.tensor_add(OT[64:128, hp, qsl], tsh[64:128, :],
                                                 qT[64:128, hp, qsl])
                    if use_bv:
                        nc.vector.tensor_scalar_add(OT[:, hp, qsl], OT[:, hp, qsl],
                                                    vecs["bv"][:, hp:hp + 1])

                def layernorm(x, y, qb, gname, bname, use_g):
                    qsl = slice(qb * 512, (qb + 1) * 512)
                    xsq = lnsq.tile([128, ET, 512], F32R)
                    for et in range(ET):
                        nc.gpsimd.tensor_mul(xsq[:, et, :], x[:, et, qsl], x[:, et, qsl])
                    mus = cps.tile([1, 512], F32, name="mus", tag="c")
                    sqs = cps.tile([1, 512], F32, name="sqs", tag="c")
                    for et in range(ET):
                        nc.tensor.matmul(mus[:], onescr[:], x[:, et, qsl],
                                         start=(et == 0), stop=(et == ET - 1))
                        nc.tensor.matmul(sqs[:], onescr[:], xsq[:, et, :],
                                         start=(et == 0), stop=(et == ET - 1))
                    mur = lnsm.tile([1, 512], F32R, name="mur", tag="mur")
                    nc.vector.tensor_copy(mur[:], mus[:])
                    musq = lnsm.tile([1, 512], F32, name="musq", tag="musq")
                    nc.vector.tensor_mul(musq[:], mur[:], mur[:])
                    var = lnsm.tile([1, 512], F32, name="var", tag="var")
                    nc.vector.tensor_sub(var[:], sqs[:], musq[:])
                    lnv = lnsm.tile([1, 512], F32, name="lnv", tag="lnv")
                    nc.scalar.activation(lnv[:], var[:], AF.Ln, bias=epst[:])
                    rstr = lnsm.tile([1, 512], F32R, name="rstr", tag="rstr")
                    nc.scalar.activation(rstr[:], lnv[:], AF.Exp, scale=-0.5)
                    muB = cps.tile([128, 512], F32, name="muB", tag="c")
                    nc.tensor.matmul(muB[:], ones1r[:], mur[:], start=True, stop=True)
                    rsB = cps.tile([128, 512], F32, name="rsB", tag="c")
                    nc.tensor.matmul(rsB[:], ones1r[:], rstr[:], start=True, stop=True)
                    for et in range(ET):
                        cen = lnsm.tile([128, 512], F32, name="cen", tag="cen")
                        nc.vector.tensor_sub(cen[:], x[:, et, qsl], muB[:])
                        dst = y[:, et, qsl]
                        nc.vector.tensor_mul(dst, cen[:], rsB[:])
                        if use_g:
                            nc.vector.tensor_scalar(
                                dst, dst, vecs[gname][:, et:et + 1],
                                vecs[bname][:, et:et + 1], ALU.mult, ALU.add)

                for qb in range(QB):
                    qsl = slice(qb * 512, (qb + 1) * 512)
                    for hp in range(2):
                        attention(hp, qb)
                    layernorm(OT, O1, qb, "g0", "b0", use_g0)
                    for et in range(ET):
                        ps = cps.tile([128, 512], F32, name="ffps", tag="c")
                        for dt in range(ET):
                            nc.tensor.matmul(
                                ps[:], w_r["Wo"][:, dt, et * 128:(et + 1) * 128],
                                O1[:, dt, qsl],
                                start=(dt == 0), stop=(dt == ET - 1))
                        ft = ffsm.tile([128, 512], F32, name="ft", tag="ft")
                        nc.vector.tensor_scalar(
                            ft[:], ps[:], vecs["bo"][:, et:et + 1] if use_bo else 0.0,
                            0.0, ALU.add, ALU.max)
                        nc.gpsimd.tensor_add(O2[:, et, qsl], O1[:, et, qsl], ft[:])
                    layernorm(O2, O3, qb, "g1", "b1", use_g1)
                    for st in range(4):
                        po = cps.tile([128, 256], F32, name="po", tag="c")
                        for dt in range(ET):
                            nc.tensor.transpose(
                                po[:, dt * 128:(dt + 1) * 128],
                                O3[:, dt, qb * 512 + st * 128:qb * 512 + (st + 1) * 128],
                                id128[:])
                        (nc.vector.tensor_copy if st % 2 == 0 else nc.scalar.copy)(
                            On[:, qb * 4 + st, :], po[:])
                    nc.sync.dma_start(
                        dO.rearrange("(g st p) d -> g p st d", p=128, st=4)[qb],
                        On[:, qb * 4:(qb + 1) * 4, :])

    nc.compile()
    return nc


def kernel(Q, K, Wq, bq, Wk, bk, Wv, bv, Wo, bo, g0, b0, g1, b1):
    Q, K = np.asarray(Q), np.asarray(K)
    ws = {n: np.ascontiguousarray(np.asarray(v), dtype=np.float32)
          for n, v in (("Wq", Wq), ("Wk", Wk), ("Wv", Wv), ("Wo", Wo))}
    vs = {n: np.ascontiguousarray(np.asarray(v), dtype=np.float32)
          for n, v in (("bq", bq), ("bk", bk), ("bv", bv), ("bo", bo),
                       ("g0", g0), ("b0", b0), ("g1", g1), ("b1", b1))}
    flags = (bool(np.any(vs["bq"])), bool(np.any(vs["bk"])),
             bool(np.any(vs["bv"])), bool(np.any(vs["bo"])),
             bool(np.any(vs["g0"] != 1.0) or np.any(vs["b0"])),
             bool(np.any(vs["g1"] != 1.0) or np.any(vs["b1"])))
    if flags not in _CACHE:
        _CACHE[flags] = _build(flags)
    nc = _CACHE[flags]

    in_maps = []
    for b in range(B):
        for half in range(2):
            m = {"Qs": np.ascontiguousarray(Q[b, half * S:(half + 1) * S], dtype=np.float32),
                 "Ks": np.ascontiguousarray(K[b], dtype=np.float32)}
            m.update(ws)
            m.update(vs)
            in_maps.append(m)

    res = run_bass_kernel_spmd(nc, in_maps, list(range(8)))
    out = np.empty((B, NQ, D), dtype=np.float32)
    for i in range(8):
        b, half = divmod(i, 2)
        out[b, half * S:(half + 1) * S] = res.results[i]["Out"]
    return out



# revision 15
# speedup vs baseline: 4.7999x; 1.0330x over previous
"""MAB (multihead attention block) TRN2 kernel.

Sharding: 8 cores = batch (4) x query-half (2). Each core computes its
[1024, 256] output slice with zero cross-core communication (K/V
projections are recomputed by the 2 cores sharing a batch).

Layout strategy: everything transposed (features on partitions) so that
- projections contract d on partitions,
- scores come out as [k, q] (exp output directly usable as A@V rhs),
- softmax denominators via a ones-row appended to V (PE does the sum),
- LN stats via ones-vector matmuls + K=1 broadcast matmuls (PE),
- FFN contracts e on partitions directly.
All matmuls run in float32r (~1.3e-4 rel err, full PE rate).
"""

import numpy as np

import concourse.bass as bass
import concourse.mybir as mybir
import concourse.tile as tile
from concourse import bacc
from concourse import masks
from concourse.bass_utils import run_bass_kernel_spmd

F32 = mybir.dt.float32
F32R = mybir.dt.float32r
AF = mybir.ActivationFunctionType
ALU = mybir.AluOpType

B, NQ, NK, D = 4, 2048, 2048, 256
H, DH = 4, 64
S = NQ // 2          # queries per core
ET = D // 128        # feature tiles
QB = S // 512        # query blocks of 512
KT = NK // 128       # key tiles of 128
KB = NK // 512       # key blocks of 512
EPS = 1e-5
SCALE = 1.0 / np.sqrt(D)

_CACHE = {}


def _build(flags):
    (use_bq, use_bk, use_bv, use_bo, use_g0, use_g1) = flags
    nc = bacc.Bacc(None, target_bir_lowering=False)

    dQ = nc.dram_tensor("Qs", [S, D], F32, kind="ExternalInput")
    dK = nc.dram_tensor("Ks", [NK, D], F32, kind="ExternalInput")
    dW = {w: nc.dram_tensor(w, [D, D], F32, kind="ExternalInput")
          for w in ("Wq", "Wk", "Wv", "Wo")}
    dV = {v: nc.dram_tensor(v, [D], F32, kind="ExternalInput")
          for v in ("bq", "bk", "bv", "bo", "g0", "b0", "g1", "b1")}
    dO = nc.dram_tensor("Out", [S, D], F32, kind="ExternalOutput")

    with tile.TileContext(nc) as tc:
        with (
            tc.tile_pool(name="const", bufs=1) as cpool,
            tc.tile_pool(name="acts", bufs=1) as apool,
            tc.tile_pool(name="big", bufs=1) as bpool,
        ):
            # ---------------- constants ----------------
            id128 = cpool.tile([128, 128], F32)
            masks.make_identity(nc, id128[:])
            w_r = {}
            for w in ("Wq", "Wk", "Wv", "Wo"):
                w_r[w] = cpool.tile([128, ET, D], F32R, name=f"wr_{w}", tag=f"wr_{w}")
            ones1 = cpool.tile([1, 128], F32)
            nc.vector.memset(ones1[:], 1.0)
            ones1r = cpool.tile([1, 128], F32R)
            nc.vector.tensor_copy(ones1r[:], ones1[:])
            onesc = cpool.tile([128, 1], F32)
            nc.vector.memset(onesc[:], 1.0 / D)
            onescr = cpool.tile([128, 1], F32R)
            nc.vector.tensor_copy(onescr[:], onesc[:])
            onesf0 = cpool.tile([128, 128], F32)
            nc.vector.memset(onesf0[:], 1.0)
            onesF = cpool.tile([128, 128], F32R)
            nc.vector.tensor_copy(onesF[:], onesf0[:])
            epst = cpool.tile([1, 1], F32)
            nc.vector.memset(epst[:], EPS)

            QT = apool.tile([128, ET, S], F32R)       # raw Q^T
            KTr = apool.tile([128, ET, NK], F32R)     # raw K^T
            qT = bpool.tile([128, ET, S], F32R)       # projected q, transposed
            kT = bpool.tile([128, ET, NK], F32R)      # projected k, transposed
            v_sb = bpool.tile([128, KT, H, 65], F32R)  # v [k, h, dh + ones col]
            OT = bpool.tile([128, ET, S], F32R)       # attention out + residual
            O1 = bpool.tile([128, ET, S], F32R)       # LN0 out
            O2 = bpool.tile([128, ET, S], F32R)       # FFN+residual out
            O3 = bpool.tile([128, ET, S], F32)        # LN1 out (transposed)
            nc.vector.tensor_copy(v_sb[:, :, :, 64:65], onesf0[:, 0:KT * H])

            # ---------- phase A: load natural, transpose on PE, project ----------
            # DMA order: weights (small, gate the projections), K chunks
            # (gate attention via kT/v_sb), Q chunks. Each K chunk is
            # transposed and projected while the next chunk transfers.
            with (
                tc.tile_pool(name="stage", bufs=1) as stpool,
                tc.tile_pool(name="tps", bufs=2, space="PSUM") as tps,
                tc.tile_pool(name="psA", bufs=4, space="PSUM") as psA,
            ):
                wn = {}
                for w in ("Wq", "Wk", "Wv", "Wo"):
                    wn[w] = stpool.tile([128, ET, D], F32, name=f"wn_{w}", tag=f"wn_{w}")
                    nc.sync.dma_start(
                        wn[w][:], dW[w].rearrange("(et p) d -> p et d", p=128))
                kn = stpool.tile([128, KT, D], F32)
                for g in range(4):
                    nc.sync.dma_start(
                        kn[:, g * 4:(g + 1) * 4, :],
                        dK.rearrange("(g st p) d -> g p st d", p=128, st=4)[g])
                qn = stpool.tile([128, 8, D], F32)
                for g in range(2):
                    nc.sync.dma_start(
                        qn[:, g * 4:(g + 1) * 4, :],
                        dQ.rearrange("(g st p) d -> g p st d", p=128, st=4)[g])
                vecs = {}
                need = {"bq": use_bq, "bk": use_bk, "bv": use_bv, "bo": use_bo,
                        "g0": use_g0, "b0": use_g0, "g1": use_g1, "b1": use_g1}
                for v in ("bq", "bk", "bv", "bo", "g0", "b0", "g1", "b1"):
                    if not need[v]:
                        continue
                    t = cpool.tile([128, ET], F32, name=f"vec_{v}", tag=f"vec_{v}")
                    nc.sync.dma_start(t[:], dV[v].rearrange("(et e) -> e et", e=128))
                    vecs[v] = t

                for w in ("Wq", "Wk", "Wv", "Wo"):
                    for dt in range(ET):
                        ps = tps.tile([128, 512], F32)
                        for et in range(ET):
                            nc.tensor.transpose(
                                ps[:, et * 128:(et + 1) * 128],
                                wn[w][:, et, dt * 128:(dt + 1) * 128], id128[:])
                        (nc.vector.tensor_copy if dt == 0 else nc.scalar.copy)(
                            w_r[w][:, dt, :], ps[:, 0:D])

                for g in range(4):          # K chunk: transpose + k/v-proj
                    for dt in range(ET):
                        ps = tps.tile([128, 512], F32)
                        for j in range(4):
                            nc.tensor.transpose(
                                ps[:, j * 128:(j + 1) * 128],
                                kn[:, g * 4 + j, dt * 128:(dt + 1) * 128], id128[:])
                        (nc.vector.tensor_copy if dt == 0 else nc.scalar.copy)(
                            KTr[:, dt, g * 512:(g + 1) * 512], ps[:])
                    for et in range(ET):
                        ps = psA.tile([128, 512], F32, name="pk", tag="pa")
                        for dt in range(ET):
                            nc.tensor.matmul(
                                ps[:], w_r["Wk"][:, dt, et * 128:(et + 1) * 128],
                                KTr[:, dt, g * 512:(g + 1) * 512],
                                start=(dt == 0), stop=(dt == ET - 1))
                        dst = kT[:, et, g * 512:(g + 1) * 512]
                        if use_bk:
                            nc.vector.tensor_scalar_add(dst, ps[:], vecs["bk"][:, et:et + 1])
                        else:
                            (nc.vector.tensor_copy if et == 0 else nc.scalar.copy)(
                                dst, ps[:])
                    for j in range(4):
                        kt = g * 4 + j
                        ps = psA.tile([128, 512], F32, name="pv", tag="pa")
                        for dt in range(ET):
                            nc.tensor.matmul(
                                ps[:, 0:256], KTr[:, dt, kt * 128:(kt + 1) * 128],
                                w_r["Wv"][:, dt, :],
                                start=(dt == 0), stop=(dt == ET - 1))
                        (nc.vector.tensor_copy if j % 2 == 0 else nc.scalar.copy)(
                            v_sb[:, kt, :, 0:64], ps[:, 0:256])

                for g in range(2):          # Q chunk: transpose + q-proj
                    for dt in range(ET):
                        ps = tps.tile([128, 512], F32)
                        for j in range(4):
                            nc.tensor.transpose(
                                ps[:, j * 128:(j + 1) * 128],
                                qn[:, g * 4 + j, dt * 128:(dt + 1) * 128], id128[:])
                        (nc.vector.tensor_copy if dt == 0 else nc.scalar.copy)(
                            QT[:, dt, g * 512:(g + 1) * 512], ps[:])
                    for et in range(ET):
                        ps = psA.tile([128, 512], F32, name="pq", tag="pa")
                        for dt in range(ET):
                            nc.tensor.matmul(
                                ps[:], w_r["Wq"][:, dt, et * 128:(et + 1) * 128],
                                QT[:, dt, g * 512:(g + 1) * 512],
                                start=(dt == 0), stop=(dt == ET - 1))
                        dst = qT[:, et, g * 512:(g + 1) * 512]
                        if use_bq:
                            nc.vector.tensor_scalar_add(dst, ps[:], vecs["bq"][:, et:et + 1])
                        else:
                            (nc.vector.tensor_copy if et == 0 else nc.scalar.copy)(
                                dst, ps[:])

            # ------------- phase B+C fused: per 512-query block -------------
            # PSUM budget (8 banks): sc 2x2 + acc 2 + shared "c" pool 2.
            On = bpool.tile([128, 8, D], F32)
            with (
                tc.tile_pool(name="scps", bufs=2, space="PSUM") as scps,
                tc.tile_pool(name="accps", bufs=1, space="PSUM") as accps,
                tc.tile_pool(name="cps", bufs=2, space="PSUM") as cps,
                tc.tile_pool(name="ut", bufs=3) as utp,
                tc.tile_pool(name="sm", bufs=2) as smp,
                tc.tile_pool(name="lnsm", bufs=2) as lnsm,
                tc.tile_pool(name="lnsq", bufs=2) as lnsq,
                tc.tile_pool(name="ffsm", bufs=2) as ffsm,
            ):
                def attention(hp, qb):
                    qsl = slice(qb * 512, (qb + 1) * 512)
                    acc = [accps.tile([65, 512], F32, name=f"acc{_h}", tag=f"acc{_h}")
                           for _h in range(2)]
                    uts = {}
                    # software-pipelined: scores/exp for kt overlap A@V for kt-1
                    for kt in range(KT + 1):
                        if kt < KT:
                            sc = scps.tile([128, 1024], F32, name="sc", tag="sc")
                            for hh in range(2):
                                off = hh * 64
                                nc.tensor.matmul(
                                    sc[:, hh * 512:(hh + 1) * 512],
                                    kT[off:off + 64, hp, kt * 128:(kt + 1) * 128],
                                    qT[off:off + 64, hp, qsl],
                                    start=True, stop=True)
                            ut = utp.tile([128, 1024], F32R)
                            nc.scalar.activation(ut[:], sc[:], AF.Exp, scale=SCALE)
                            uts[kt] = ut
                        if kt >= 1:
                            utp_ = uts.pop(kt - 1)
                            for hh in range(2):
                                h = hp * 2 + hh
                                nc.tensor.matmul(
                                    acc[hh][:],
                                    v_sb[:, kt - 1, h, :],
                                    utp_[:, hh * 512:(hh + 1) * 512],
                                    start=(kt - 1 == 0), stop=(kt - 1 == KT - 1))
                    for hh in range(2):
                        dcp = smp.tile([65, 512], F32, name=f"dcp{hh}", tag="dcp")
                        nc.vector.tensor_copy(dcp[64:65, :], acc[hh][64:65, :])
                        den0 = smp.tile([1, 512], F32, name=f"den0{hh}", tag="den0")
                        nc.sync.dma_start(den0[0:1, :], dcp[64:65, :])
                        rec = smp.tile([1, 512], F32, name=f"rec{hh}", tag="rec")
                        nc.vector.reciprocal_approx_fast(out=rec[:], in_=den0[:])
                        recr = smp.tile([1, 512], F32R, name=f"recr{hh}", tag="recr")
                        nc.vector.tensor_copy(recr[:], rec[:])
                        recB = cps.tile([64, 512], F32, name=f"recB{hh}", tag="c")
                        nc.tensor.matmul(recB[:], onesF[0:1, 0:64], recr[:],
                                         start=True, stop=True)
                        recS = smp.tile([64, 512], F32, name=f"recS{hh}", tag="recS")
                        nc.vector.tensor_copy(recS[:], recB[:])
                        tmp = smp.tile([64, 512], F32, name=f"tmp{hh}", tag="tmp")
                        nc.vector.tensor_mul(tmp[:], acc[hh][0:64, :], recS[:])
                        if hh == 0:
                            nc.gpsimd.tensor_add(OT[0:64, hp, qsl], tmp[:],
                                                 qT[0:64, hp, qsl])
                        else:
                            tsh = smp.tile([128, 512], F32, name="tsh", tag="tsh")
                            nc.sync.dma_start(tsh[64:128, :], tmp[:])
                            nc.gpsimd.tensor_add(OT[64:128, hp, qsl], tsh[64:128, :],
                                                 qT[64:128, hp, qsl])
                    if use_bv:
                        nc.vector.tensor_scalar_add(OT[:, hp, qsl], OT[:, hp, qsl],
                                                    vecs["bv"][:, hp:hp + 1])

                def layernorm(x, y, qb, gname, bname, use_g):
                    qsl = slice(qb * 512, (qb + 1) * 512)
                    xsq = lnsq.tile([128, ET, 512], F32R)
                    for et in range(ET):
                        nc.gpsimd.tensor_mul(xsq[:, et, :], x[:, et, qsl], x[:, et, qsl])
                    mus = cps.tile([1, 512], F32, name="mus", tag="c")
                    sqs = cps.tile([1, 512], F32, name="sqs", tag="c")
                    for et in range(ET):
                        nc.tensor.matmul(mus[:], onescr[:], x[:, et, qsl],
                                         start=(et == 0), stop=(et == ET - 1))
                        nc.tensor.matmul(sqs[:], onescr[:], xsq[:, et, :],
                                         start=(et == 0), stop=(et == ET - 1))
                    mur = lnsm.tile([1, 512], F32R, name="mur", tag="mur")
                    nc.vector.tensor_copy(mur[:], mus[:])
                    musq = lnsm.tile([1, 512], F32, name="musq", tag="musq")
                    nc.vector.tensor_mul(musq[:], mur[:], mur[:])
                    var = lnsm.tile([1, 512], F32, name="var", tag="var")
                    nc.vector.tensor_sub(var[:], sqs[:], musq[:])
                    lnv = lnsm.tile([1, 512], F32, name="lnv", tag="lnv")
                    nc.scalar.activation(lnv[:], var[:], AF.Ln, bias=epst[:])
                    rstr = lnsm.tile([1, 512], F32R, name="rstr", tag="rstr")
                    nc.scalar.activation(rstr[:], lnv[:], AF.Exp, scale=-0.5)
                    muB = cps.tile([128, 512], F32, name="muB", tag="c")
                    nc.tensor.matmul(muB[:], ones1r[:], mur[:], start=True, stop=True)
                    rsB = cps.tile([128, 512], F32, name="rsB", tag="c")
                    nc.tensor.matmul(rsB[:], ones1r[:], rstr[:], start=True, stop=True)
                    for et in range(ET):
                        cen = lnsm.tile([128, 512], F32, name="cen", tag="cen")
                        nc.vector.tensor_sub(cen[:], x[:, et, qsl], muB[:])
                        dst = y[:, et, qsl]
                        nc.vector.tensor_mul(dst, cen[:], rsB[:])
                        if use_g:
                            nc.vector.tensor_scalar(
                                dst, dst, vecs[gname][:, et:et + 1],
                                vecs[bname][:, et:et + 1], ALU.mult, ALU.add)

                for qb in range(QB):
                    qsl = slice(qb * 512, (qb + 1) * 512)
                    for hp in range(2):
                        attention(hp, qb)
                    layernorm(OT, O1, qb, "g0", "b0", use_g0)
                    for et in range(ET):
                        ps = cps.tile([128, 512], F32, name="ffps", tag="c")
                        for dt in range(ET):
                            nc.tensor.matmul(
                                ps[:], w_r["Wo"][:, dt, et * 128:(et + 1) * 128],
                                O1[:, dt, qsl],
                                start=(dt == 0), stop=(dt == ET - 1))
                        ft = ffsm.tile([128, 512], F32, name="ft", tag="ft")
                        nc.vector.tensor_scalar(
                            ft[:], ps[:], vecs["bo"][:, et:et + 1] if use_bo else 0.0,
                            0.0, ALU.add, ALU.max)
                        nc.gpsimd.tensor_add(O2[:, et, qsl], O1[:, et, qsl], ft[:])
                    layernorm(O2, O3, qb, "g1", "b1", use_g1)
                    for st in range(4):
                        po = cps.tile([128, 256], F32, name="po", tag="c")
                        for dt in range(ET):
                            nc.tensor.transpose(
                                po[:, dt * 128:(dt + 1) * 128],
                                O3[:, dt, qb * 512 + st * 128:qb * 512 + (st + 1) * 128],
                                id128[:])
                        (nc.vector.tensor_copy if st % 2 == 0 else nc.scalar.copy)(
                            On[:, qb * 4 + st, :], po[:])
                    nc.sync.dma_start(
                        dO.rearrange("(g st p) d -> g p st d", p=128, st=4)[qb],
                        On[:, qb * 4:(qb + 1) * 4, :])

    nc.compile()
    return nc


def kernel(Q, K, Wq, bq, Wk, bk, Wv, bv, Wo, bo, g0, b0, g1, b1):
    Q, K = np.asarray(Q), np.asarray(K)
    ws = {n: np.ascontiguousarray(np.asarray(v), dtype=np.float32)
          for n, v in (("Wq", Wq), ("Wk", Wk), ("Wv", Wv), ("Wo", Wo))}
    vs = {n: np.ascontiguousarray(np.asarray(v), dtype=np.float32)
          for n, v in (("bq", bq), ("bk", bk), ("bv", bv), ("bo", bo),
                       ("g0", g0), ("b0", b0), ("g1", g1), ("b1", b1))}
    flags = (bool(np.any(vs["bq"])), bool(np.any(vs["bk"])),
             bool(np.any(vs["bv"])), bool(np.any(vs["bo"])),
             bool(np.any(vs["g0"] != 1.0) or np.any(vs["b0"])),
             bool(np.any(vs["g1"] != 1.0) or np.any(vs["b1"])))
    if flags not in _CACHE:
        _CACHE[flags] = _build(flags)
    nc = _CACHE[flags]

    in_maps = []
    for b in range(B):
        for half in range(2):
            m = {"Qs": np.ascontiguousarray(Q[b, half * S:(half + 1) * S], dtype=np.float32),
                 "Ks": np.ascontiguousarray(K[b], dtype=np.float32)}
            m.update(ws)
            m.update(vs)
            in_maps.append(m)

    res = run_bass_kernel_spmd(nc, in_maps, list(range(8)))
    out = np.empty((B, NQ, D), dtype=np.float32)
    for i in range(8):
        b, half = divmod(i, 2)
        out[b, half * S:(half + 1) * S] = res.results[i]["Out"]
    return out



# revision 16
# speedup vs baseline: 5.0555x; 1.0533x over previous
"""MAB (multihead attention block) TRN2 kernel.

Sharding: 8 cores = batch (4) x query-half (2). Each core computes its
[1024, 256] output slice with zero cross-core communication (K/V
projections are recomputed by the 2 cores sharing a batch).

Layout strategy: everything transposed (features on partitions) so that
- projections contract d on partitions,
- scores come out as [k, q] (exp output directly usable as A@V rhs),
- softmax denominators via a ones-row appended to V (PE does the sum),
- LN stats via ones-vector matmuls + K=1 broadcast matmuls (PE),
- FFN contracts e on partitions directly.
All matmuls run in float32r (~1.3e-4 rel err, full PE rate).
"""

import numpy as np

import concourse.bass as bass
import concourse.mybir as mybir
import concourse.tile as tile
from concourse import bacc
from concourse import masks
from concourse.bass_utils import run_bass_kernel_spmd

F32 = mybir.dt.float32
F32R = mybir.dt.float32r
AF = mybir.ActivationFunctionType
ALU = mybir.AluOpType

B, NQ, NK, D = 4, 2048, 2048, 256
H, DH = 4, 64
S = NQ // 2          # queries per core
ET = D // 128        # feature tiles
QB = S // 512        # query blocks of 512
KT = NK // 128       # key tiles of 128
KB = NK // 512       # key blocks of 512
EPS = 1e-5
SCALE = 1.0 / np.sqrt(D)

_CACHE = {}


def _build(flags):
    (use_bq, use_bk, use_bv, use_bo, use_g0, use_g1) = flags
    nc = bacc.Bacc(None, target_bir_lowering=False)

    dQ = nc.dram_tensor("Qs", [S, D], F32, kind="ExternalInput")
    dK = nc.dram_tensor("Ks", [NK, D], F32, kind="ExternalInput")
    dW = {w: nc.dram_tensor(w, [D, D], F32, kind="ExternalInput")
          for w in ("Wq", "Wk", "Wv", "Wo")}
    dV = {v: nc.dram_tensor(v, [D], F32, kind="ExternalInput")
          for v in ("bq", "bk", "bv", "bo", "g0", "b0", "g1", "b1")}
    dO = nc.dram_tensor("Out", [S, D], F32, kind="ExternalOutput")

    with tile.TileContext(nc) as tc:
        with (
            tc.tile_pool(name="const", bufs=1) as cpool,
            tc.tile_pool(name="acts", bufs=1) as apool,
            tc.tile_pool(name="big", bufs=1) as bpool,
        ):
            # ---------------- constants ----------------
            id128 = cpool.tile([128, 128], F32)
            masks.make_identity(nc, id128[:])
            w_r = {}
            for w in ("Wq", "Wk", "Wv", "Wo"):
                w_r[w] = cpool.tile([128, ET, D], F32R, name=f"wr_{w}", tag=f"wr_{w}")
            ones1 = cpool.tile([1, 128], F32)
            nc.vector.memset(ones1[:], 1.0)
            ones1r = cpool.tile([1, 128], F32R)
            nc.vector.tensor_copy(ones1r[:], ones1[:])
            onesc = cpool.tile([128, 1], F32)
            nc.vector.memset(onesc[:], 1.0 / D)
            onescr = cpool.tile([128, 1], F32R)
            nc.vector.tensor_copy(onescr[:], onesc[:])
            onesf0 = cpool.tile([128, 128], F32)
            nc.vector.memset(onesf0[:], 1.0)
            onesF = cpool.tile([128, 128], F32R)
            nc.vector.tensor_copy(onesF[:], onesf0[:])
            epst = cpool.tile([1, 1], F32)
            nc.vector.memset(epst[:], EPS)

            QT = apool.tile([128, ET, S], F32R)       # raw Q^T
            KTr = apool.tile([128, ET, NK], F32R)     # raw K^T
            qT = bpool.tile([128, ET, S], F32R)       # projected q, transposed
            kT = bpool.tile([128, ET, NK], F32R)      # projected k, transposed
            v_sb = bpool.tile([128, KT, H, 65], F32R)  # v [k, h, dh + ones col]
            OT = bpool.tile([128, ET, S], F32R)       # attention out + residual
            O1 = bpool.tile([128, ET, S], F32R)       # LN0 out
            O2 = bpool.tile([128, ET, S], F32R)       # FFN+residual out
            O3 = bpool.tile([128, ET, S], F32)        # LN1 out (transposed)
            nc.vector.tensor_copy(v_sb[:, :, :, 64:65], onesf0[:, 0:KT * H])

            # ---------- phase A: load natural, transpose on PE, project ----------
            # DMA order: weights (small, gate the projections), K chunks
            # (gate attention via kT/v_sb), Q chunks. Each K chunk is
            # transposed and projected while the next chunk transfers.
            with (
                tc.tile_pool(name="stage", bufs=1) as stpool,
                tc.tile_pool(name="tps", bufs=2, space="PSUM") as tps,
                tc.tile_pool(name="psA", bufs=4, space="PSUM") as psA,
            ):
                wn = {}
                for w in ("Wq", "Wk", "Wv", "Wo"):
                    wn[w] = stpool.tile([128, ET, D], F32, name=f"wn_{w}", tag=f"wn_{w}")
                    nc.sync.dma_start(
                        wn[w][:], dW[w].rearrange("(et p) d -> p et d", p=128))
                kn = stpool.tile([128, KT, D], F32)
                for g in range(4):
                    nc.sync.dma_start(
                        kn[:, g * 4:(g + 1) * 4, :],
                        dK.rearrange("(g st p) d -> g p st d", p=128, st=4)[g])
                qn = stpool.tile([128, 8, D], F32)
                for g in range(2):
                    nc.sync.dma_start(
                        qn[:, g * 4:(g + 1) * 4, :],
                        dQ.rearrange("(g st p) d -> g p st d", p=128, st=4)[g])
                vecs = {}
                need = {"bq": use_bq, "bk": use_bk, "bv": use_bv, "bo": use_bo,
                        "g0": use_g0, "b0": use_g0, "g1": use_g1, "b1": use_g1}
                for v in ("bq", "bk", "bv", "bo", "g0", "b0", "g1", "b1"):
                    if not need[v]:
                        continue
                    t = cpool.tile([128, ET], F32, name=f"vec_{v}", tag=f"vec_{v}")
                    nc.sync.dma_start(t[:], dV[v].rearrange("(et e) -> e et", e=128))
                    vecs[v] = t

                for w in ("Wq", "Wk", "Wv", "Wo"):
                    for dt in range(ET):
                        ps = tps.tile([128, 512], F32)
                        for et in range(ET):
                            nc.tensor.transpose(
                                ps[:, et * 128:(et + 1) * 128],
                                wn[w][:, et, dt * 128:(dt + 1) * 128], id128[:])
                        (nc.vector.tensor_copy if dt == 0 else nc.scalar.copy)(
                            w_r[w][:, dt, :], ps[:, 0:D])

                for g in range(4):          # K chunk: transpose + k/v-proj
                    for dt in range(ET):
                        ps = tps.tile([128, 512], F32)
                        for j in range(4):
                            nc.tensor.transpose(
                                ps[:, j * 128:(j + 1) * 128],
                                kn[:, g * 4 + j, dt * 128:(dt + 1) * 128], id128[:])
                        (nc.vector.tensor_copy if dt == 0 else nc.scalar.copy)(
                            KTr[:, dt, g * 512:(g + 1) * 512], ps[:])
                    for et in range(ET):
                        ps = psA.tile([128, 512], F32, name="pk", tag="pa")
                        for dt in range(ET):
                            nc.tensor.matmul(
                                ps[:], w_r["Wk"][:, dt, et * 128:(et + 1) * 128],
                                KTr[:, dt, g * 512:(g + 1) * 512],
                                start=(dt == 0), stop=(dt == ET - 1))
                        dst = kT[:, et, g * 512:(g + 1) * 512]
                        if use_bk:
                            nc.vector.tensor_scalar_add(dst, ps[:], vecs["bk"][:, et:et + 1])
                        else:
                            (nc.vector.tensor_copy if et == 0 else nc.scalar.copy)(
                                dst, ps[:])
                    for j in range(4):
                        kt = g * 4 + j
                        ps = psA.tile([128, 512], F32, name="pv", tag="pa")
                        for dt in range(ET):
                            nc.tensor.matmul(
                                ps[:, 0:256], KTr[:, dt, kt * 128:(kt + 1) * 128],
                                w_r["Wv"][:, dt, :],
                                start=(dt == 0), stop=(dt == ET - 1))
                        (nc.vector.tensor_copy if j % 2 == 0 else nc.scalar.copy)(
                            v_sb[:, kt, :, 0:64], ps[:, 0:256])

                for g in range(2):          # Q chunk: transpose + q-proj
                    for dt in range(ET):
                        ps = tps.tile([128, 512], F32)
                        for j in range(4):
                            nc.tensor.transpose(
                                ps[:, j * 128:(j + 1) * 128],
                                qn[:, g * 4 + j, dt * 128:(dt + 1) * 128], id128[:])
                        (nc.vector.tensor_copy if dt == 0 else nc.scalar.copy)(
                            QT[:, dt, g * 512:(g + 1) * 512], ps[:])
                    for et in range(ET):
                        ps = psA.tile([128, 512], F32, name="pq", tag="pa")
                        for dt in range(ET):
                            nc.tensor.matmul(
                                ps[:], w_r["Wq"][:, dt, et * 128:(et + 1) * 128],
                                QT[:, dt, g * 512:(g + 1) * 512],
                                start=(dt == 0), stop=(dt == ET - 1))
                        dst = qT[:, et, g * 512:(g + 1) * 512]
                        if use_bq:
                            nc.vector.tensor_scalar_add(dst, ps[:], vecs["bq"][:, et:et + 1])
                        else:
                            (nc.vector.tensor_copy if et == 0 else nc.scalar.copy)(
                                dst, ps[:])

            # ------------- phase B: attention (all blocks) -------------
            On = bpool.tile([128, 8, D], F32)
            with (
                tc.tile_pool(name="ut", bufs=3) as utp,
                tc.tile_pool(name="sm", bufs=2) as smp,
            ):
                with (
                    tc.tile_pool(name="scps", bufs=2, space="PSUM") as scps,
                    tc.tile_pool(name="accps", bufs=1, space="PSUM") as accps,
                    tc.tile_pool(name="bcps", bufs=1, space="PSUM") as bcps,
                ):
                    def attention(hp, qb):
                        qsl = slice(qb * 512, (qb + 1) * 512)
                        acc = [accps.tile([65, 512], F32, name=f"acc{_h}", tag=f"acc{_h}")
                               for _h in range(2)]
                        uts = {}
                        # software-pipelined: scores/exp(kt) overlap A@V(kt-1)
                        for kt in range(KT + 1):
                            if kt < KT:
                                sc = scps.tile([128, 1024], F32, name="sc", tag="sc")
                                for hh in range(2):
                                    off = hh * 64
                                    nc.tensor.matmul(
                                        sc[:, hh * 512:(hh + 1) * 512],
                                        kT[off:off + 64, hp, kt * 128:(kt + 1) * 128],
                                        qT[off:off + 64, hp, qsl],
                                        start=True, stop=True)
                                ut = utp.tile([128, 1024], F32R)
                                nc.scalar.activation(ut[:], sc[:], AF.Exp, scale=SCALE)
                                uts[kt] = ut
                            if kt >= 1:
                                utp_ = uts.pop(kt - 1)
                                for hh in range(2):
                                    h = hp * 2 + hh
                                    nc.tensor.matmul(
                                        acc[hh][:],
                                        v_sb[:, kt - 1, h, :],
                                        utp_[:, hh * 512:(hh + 1) * 512],
                                        start=(kt - 1 == 0), stop=(kt - 1 == KT - 1))
                        for hh in range(2):
                            dcp = smp.tile([65, 512], F32, name=f"dcp{hh}", tag="dcp")
                            nc.vector.tensor_copy(dcp[64:65, :], acc[hh][64:65, :])
                            den0 = smp.tile([1, 512], F32, name=f"den0{hh}", tag="den0")
                            nc.sync.dma_start(den0[0:1, :], dcp[64:65, :])
                            rec = smp.tile([1, 512], F32, name=f"rec{hh}", tag="rec")
                            nc.vector.reciprocal_approx_fast(out=rec[:], in_=den0[:])
                            recr = smp.tile([1, 512], F32R, name=f"recr{hh}", tag="recr")
                            nc.vector.tensor_copy(recr[:], rec[:])
                            recB = bcps.tile([64, 512], F32, name=f"recB{hh}", tag="recB")
                            nc.tensor.matmul(recB[:], onesF[0:1, 0:64], recr[:],
                                             start=True, stop=True)
                            recS = smp.tile([64, 512], F32, name=f"recS{hh}", tag="recS")
                            nc.vector.tensor_copy(recS[:], recB[:])
                            tmp = smp.tile([64, 512], F32, name=f"tmp{hh}", tag="tmp")
                            nc.vector.tensor_mul(tmp[:], acc[hh][0:64, :], recS[:])
                            if hh == 0:
                                nc.gpsimd.tensor_add(OT[0:64, hp, qsl], tmp[:],
                                                     qT[0:64, hp, qsl])
                            else:
                                tsh = smp.tile([128, 512], F32, name="tsh", tag="tsh")
                                nc.sync.dma_start(tsh[64:128, :], tmp[:])
                                nc.gpsimd.tensor_add(OT[64:128, hp, qsl], tsh[64:128, :],
                                                     qT[64:128, hp, qsl])
                        if use_bv:
                            nc.vector.tensor_scalar_add(OT[:, hp, qsl], OT[:, hp, qsl],
                                                        vecs["bv"][:, hp:hp + 1])

                    for qb in range(QB):
                        for hp in range(2):
                            attention(hp, qb)

                # ------------- phase C: LN0 -> FFN -> LN1 -> store -------------
                with (
                    tc.tile_pool(name="lnps", bufs=1, space="PSUM") as lnps,
                    tc.tile_pool(name="lnbc", bufs=1, space="PSUM") as lnbc,
                    tc.tile_pool(name="ffps", bufs=2, space="PSUM") as ffps,
                    tc.tile_pool(name="ops", bufs=2, space="PSUM") as ops,
                    tc.tile_pool(name="lnsm", bufs=2) as lnsm,
                    tc.tile_pool(name="lnsq", bufs=2) as lnsq,
                    tc.tile_pool(name="ffsm", bufs=2) as ffsm,
                ):
                    def layernorm(x, y, qb, gname, bname, use_g):
                        qsl = slice(qb * 512, (qb + 1) * 512)
                        xsq = lnsq.tile([128, ET, 512], F32R)
                        for et in range(ET):
                            nc.gpsimd.tensor_mul(xsq[:, et, :], x[:, et, qsl], x[:, et, qsl])
                        mus = lnps.tile([1, 512], F32, name="mus", tag="mus")
                        sqs = lnps.tile([1, 512], F32, name="sqs", tag="sqs")
                        for et in range(ET):
                            nc.tensor.matmul(mus[:], onescr[:], x[:, et, qsl],
                                             start=(et == 0), stop=(et == ET - 1))
                            nc.tensor.matmul(sqs[:], onescr[:], xsq[:, et, :],
                                             start=(et == 0), stop=(et == ET - 1))
                        mur = lnsm.tile([1, 512], F32R, name="mur", tag="mur")
                        nc.vector.tensor_copy(mur[:], mus[:])
                        musq = lnsm.tile([1, 512], F32, name="musq", tag="musq")
                        nc.vector.tensor_mul(musq[:], mur[:], mur[:])
                        var = lnsm.tile([1, 512], F32, name="var", tag="var")
                        nc.vector.tensor_sub(var[:], sqs[:], musq[:])
                        lnv = lnsm.tile([1, 512], F32, name="lnv", tag="lnv")
                        nc.scalar.activation(lnv[:], var[:], AF.Ln, bias=epst[:])
                        rstr = lnsm.tile([1, 512], F32R, name="rstr", tag="rstr")
                        nc.scalar.activation(rstr[:], lnv[:], AF.Exp, scale=-0.5)
                        muB = lnbc.tile([128, 512], F32, name="muB", tag="muB")
                        nc.tensor.matmul(muB[:], ones1r[:], mur[:], start=True, stop=True)
                        rsB = lnbc.tile([128, 512], F32, name="rsB", tag="rsB")
                        nc.tensor.matmul(rsB[:], ones1r[:], rstr[:], start=True, stop=True)
                        for et in range(ET):
                            cen = lnsm.tile([128, 512], F32, name="cen", tag="cen")
                            nc.vector.tensor_sub(cen[:], x[:, et, qsl], muB[:])
                            dst = y[:, et, qsl]
                            nc.vector.tensor_mul(dst, cen[:], rsB[:])
                            if use_g:
                                nc.vector.tensor_scalar(
                                    dst, dst, vecs[gname][:, et:et + 1],
                                    vecs[bname][:, et:et + 1], ALU.mult, ALU.add)

                    for qb in range(QB):
                        layernorm(OT, O1, qb, "g0", "b0", use_g0)
                    for qb in range(QB):
                        qsl = slice(qb * 512, (qb + 1) * 512)
                        for et in range(ET):
                            ps = ffps.tile([128, 512], F32)
                            for dt in range(ET):
                                nc.tensor.matmul(
                                    ps[:], w_r["Wo"][:, dt, et * 128:(et + 1) * 128],
                                    O1[:, dt, qsl],
                                    start=(dt == 0), stop=(dt == ET - 1))
                            ft = ffsm.tile([128, 512], F32, name="ft", tag="ft")
                            nc.vector.tensor_scalar(
                                ft[:], ps[:], vecs["bo"][:, et:et + 1] if use_bo else 0.0,
                                0.0, ALU.add, ALU.max)
                            nc.gpsimd.tensor_add(O2[:, et, qsl], O1[:, et, qsl], ft[:])
                    for qb in range(QB):
                        layernorm(O2, O3, qb, "g1", "b1", use_g1)
                    for qb in range(QB):
                        for st in range(4):
                            po = ops.tile([128, 256], F32)
                            for dt in range(ET):
                                nc.tensor.transpose(
                                    po[:, dt * 128:(dt + 1) * 128],
                                    O3[:, dt, qb * 512 + st * 128:qb * 512 + (st + 1) * 128],
                                    id128[:])
                            (nc.vector.tensor_copy if st % 2 == 0 else nc.scalar.copy)(
                                On[:, qb * 4 + st, :], po[:])
                        nc.sync.dma_start(
                            dO.rearrange("(g st p) d -> g p st d", p=128, st=4)[qb],
                            On[:, qb * 4:(qb + 1) * 4, :])

    nc.compile()
    return nc


def kernel(Q, K, Wq, bq, Wk, bk, Wv, bv, Wo, bo, g0, b0, g1, b1):
    Q, K = np.asarray(Q), np.asarray(K)
    ws = {n: np.ascontiguousarray(np.asarray(v), dtype=np.float32)
          for n, v in (("Wq", Wq), ("Wk", Wk), ("Wv", Wv), ("Wo", Wo))}
    vs = {n: np.ascontiguousarray(np.asarray(v), dtype=np.float32)
          for n, v in (("bq", bq), ("bk", bk), ("bv", bv), ("bo", bo),
                       ("g0", g0), ("b0", b0), ("g1", g1), ("b1", b1))}
    flags = (bool(np.any(vs["bq"])), bool(np.any(vs["bk"])),
             bool(np.any(vs["bv"])), bool(np.any(vs["bo"])),
             bool(np.any(vs["g0"] != 1.0) or np.any(vs["b0"])),
             bool(np.any(vs["g1"] != 1.0) or np.any(vs["b1"])))
    if flags not in _CACHE:
        _CACHE[flags] = _build(flags)
    nc = _CACHE[flags]

    in_maps = []
    for b in range(B):
        for half in range(2):
            m = {"Qs": np.ascontiguousarray(Q[b, half * S:(half + 1) * S], dtype=np.float32),
                 "Ks": np.ascontiguousarray(K[b], dtype=np.float32)}
            m.update(ws)
            m.update(vs)
            in_maps.append(m)

    res = run_bass_kernel_spmd(nc, in_maps, list(range(8)))
    out = np.empty((B, NQ, D), dtype=np.float32)
    for i in range(8):
        b, half = divmod(i, 2)
        out[b, half * S:(half + 1) * S] = res.results[i]["Out"]
    return out



# revision 17
# speedup vs baseline: 5.5777x; 1.1033x over previous
"""MAB (multihead attention block) TRN2 kernel.

Sharding: 8 cores = batch (4) x query-half (2). Each core computes its
[1024, 256] output slice with zero cross-core communication (K/V
projections are recomputed by the 2 cores sharing a batch).

Layout strategy: everything transposed (features on partitions) so that
- projections contract d on partitions,
- scores come out as [k, q] (exp output directly usable as A@V rhs),
- softmax denominators via a ones-row appended to V (PE does the sum),
- LN stats via ones-vector matmuls + K=1 broadcast matmuls (PE),
- FFN contracts e on partitions directly.
All matmuls run in float32r (~1.3e-4 rel err, full PE rate).
"""

import numpy as np

import concourse.bass as bass
import concourse.mybir as mybir
import concourse.tile as tile
from concourse import bacc
from concourse import masks
from concourse.bass_utils import run_bass_kernel_spmd

F32 = mybir.dt.float32
F32R = mybir.dt.float32r
AF = mybir.ActivationFunctionType
ALU = mybir.AluOpType

B, NQ, NK, D = 4, 2048, 2048, 256
H, DH = 4, 64
S = NQ // 2          # queries per core
ET = D // 128        # feature tiles
QB = S // 512        # query blocks of 512
KT = NK // 128       # key tiles of 128
KB = NK // 512       # key blocks of 512
EPS = 1e-5
SCALE = 1.0 / np.sqrt(D)

_CACHE = {}


def _build(flags):
    (use_bq, use_bk, use_bv, use_bo, use_g0, use_g1) = flags
    nc = bacc.Bacc(None, target_bir_lowering=False)

    dQ = nc.dram_tensor("Qs", [S, D], F32, kind="ExternalInput")
    dK = nc.dram_tensor("Ks", [NK, D], F32, kind="ExternalInput")
    dW = {w: nc.dram_tensor(w, [D, D], F32, kind="ExternalInput")
          for w in ("Wq", "Wk", "Wv", "Wo")}
    dV = {v: nc.dram_tensor(v, [D], F32, kind="ExternalInput")
          for v in ("bq", "bk", "bv", "bo", "g0", "b0", "g1", "b1")}
    dO = nc.dram_tensor("Out", [S, D], F32, kind="ExternalOutput")

    with tile.TileContext(nc) as tc:
        with (
            tc.tile_pool(name="const", bufs=1) as cpool,
            tc.tile_pool(name="acts", bufs=1) as apool,
            tc.tile_pool(name="big", bufs=1) as bpool,
        ):
            # ---------------- constants ----------------
            id128 = cpool.tile([128, 128], F32)
            masks.make_identity(nc, id128[:])
            w_r = {}
            for w in ("Wq", "Wk", "Wv", "Wo"):
                w_r[w] = cpool.tile([128, ET, D], F32R, name=f"wr_{w}", tag=f"wr_{w}")
            ones1 = cpool.tile([1, 128], F32)
            nc.vector.memset(ones1[:], 1.0)
            ones1r = cpool.tile([1, 128], F32R)
            nc.vector.tensor_copy(ones1r[:], ones1[:])
            onesc = cpool.tile([128, 1], F32)
            nc.vector.memset(onesc[:], 1.0 / D)
            onescr = cpool.tile([128, 1], F32R)
            nc.vector.tensor_copy(onescr[:], onesc[:])
            onesf0 = cpool.tile([128, 128], F32)
            nc.vector.memset(onesf0[:], 1.0)
            onesF = cpool.tile([128, 128], F32R)
            nc.vector.tensor_copy(onesF[:], onesf0[:])
            epst = cpool.tile([1, 1], F32)
            nc.vector.memset(epst[:], EPS)

            QT = apool.tile([128, ET, S], F32R)       # raw Q^T
            KTr = apool.tile([128, ET, NK], F32R)     # raw K^T
            qT = bpool.tile([128, ET, S], F32R)       # projected q, transposed
            kT = bpool.tile([128, ET, NK], F32R)      # projected k, transposed
            v_sb = bpool.tile([128, KT, H, 65], F32R)  # v [k, h, dh + ones col]
            OT = bpool.tile([128, ET, S], F32R)       # attention out + residual
            O1 = bpool.tile([128, ET, S], F32R)       # LN0 out
            O2 = bpool.tile([128, ET, S], F32R)       # FFN+residual out
            O3 = bpool.tile([128, ET, S], F32)        # LN1 out (transposed)
            nc.vector.tensor_copy(v_sb[:, :, :, 64:65], onesf0[:, 0:KT * H])

            # ---------- phase A: load natural, transpose on PE, project ----------
            # DMA order: weights (small, gate the projections), K chunks
            # (gate attention via kT/v_sb), Q chunks. Each K chunk is
            # transposed and projected while the next chunk transfers.
            with (
                tc.tile_pool(name="stage", bufs=1) as stpool,
                tc.tile_pool(name="tps", bufs=2, space="PSUM") as tps,
                tc.tile_pool(name="psA", bufs=4, space="PSUM") as psA,
            ):
                wn = {}
                for w in ("Wq", "Wk", "Wv", "Wo"):
                    wn[w] = stpool.tile([128, ET, D], F32, name=f"wn_{w}", tag=f"wn_{w}")
                    nc.sync.dma_start(
                        wn[w][:], dW[w].rearrange("(et p) d -> p et d", p=128))
                kn = stpool.tile([128, KT, D], F32)
                for g in range(4):
                    nc.sync.dma_start(
                        kn[:, g * 4:(g + 1) * 4, :],
                        dK.rearrange("(g st p) d -> g p st d", p=128, st=4)[g])
                qn = stpool.tile([128, 8, D], F32)
                for g in range(2):
                    nc.sync.dma_start(
                        qn[:, g * 4:(g + 1) * 4, :],
                        dQ.rearrange("(g st p) d -> g p st d", p=128, st=4)[g])
                vecs = {}
                need = {"bq": use_bq, "bk": use_bk, "bv": use_bv, "bo": use_bo,
                        "g0": use_g0, "b0": use_g0, "g1": use_g1, "b1": use_g1}
                for v in ("bq", "bk", "bv", "bo", "g0", "b0", "g1", "b1"):
                    if not need[v]:
                        continue
                    t = cpool.tile([128, ET], F32, name=f"vec_{v}", tag=f"vec_{v}")
                    nc.sync.dma_start(t[:], dV[v].rearrange("(et e) -> e et", e=128))
                    vecs[v] = t

                for w in ("Wq", "Wk", "Wv", "Wo"):
                    for dt in range(ET):
                        ps = tps.tile([128, 512], F32)
                        for et in range(ET):
                            nc.tensor.transpose(
                                ps[:, et * 128:(et + 1) * 128],
                                wn[w][:, et, dt * 128:(dt + 1) * 128], id128[:])
                        (nc.vector.tensor_copy if dt == 0 else nc.scalar.copy)(
                            w_r[w][:, dt, :], ps[:, 0:D])

                for g in range(4):          # K chunk: transpose + k/v-proj
                    for dt in range(ET):
                        ps = tps.tile([128, 512], F32)
                        for j in range(4):
                            nc.tensor.transpose(
                                ps[:, j * 128:(j + 1) * 128],
                                kn[:, g * 4 + j, dt * 128:(dt + 1) * 128], id128[:])
                        (nc.vector.tensor_copy if dt == 0 else nc.scalar.copy)(
                            KTr[:, dt, g * 512:(g + 1) * 512], ps[:])
                    for et in range(ET):
                        ps = psA.tile([128, 512], F32, name="pk", tag="pa")
                        for dt in range(ET):
                            nc.tensor.matmul(
                                ps[:], w_r["Wk"][:, dt, et * 128:(et + 1) * 128],
                                KTr[:, dt, g * 512:(g + 1) * 512],
                                start=(dt == 0), stop=(dt == ET - 1))
                        dst = kT[:, et, g * 512:(g + 1) * 512]
                        if use_bk:
                            nc.vector.tensor_scalar_add(dst, ps[:], vecs["bk"][:, et:et + 1])
                        else:
                            (nc.vector.tensor_copy if et == 0 else nc.scalar.copy)(
                                dst, ps[:])
                    for j in range(4):
                        kt = g * 4 + j
                        ps = psA.tile([128, 512], F32, name="pv", tag="pa")
                        for dt in range(ET):
                            nc.tensor.matmul(
                                ps[:, 0:256], KTr[:, dt, kt * 128:(kt + 1) * 128],
                                w_r["Wv"][:, dt, :],
                                start=(dt == 0), stop=(dt == ET - 1))
                        (nc.vector.tensor_copy if j % 2 == 0 else nc.scalar.copy)(
                            v_sb[:, kt, :, 0:64], ps[:, 0:256])

                for g in range(2):          # Q chunk: transpose + q-proj
                    for dt in range(ET):
                        ps = tps.tile([128, 512], F32)
                        for j in range(4):
                            nc.tensor.transpose(
                                ps[:, j * 128:(j + 1) * 128],
                                qn[:, g * 4 + j, dt * 128:(dt + 1) * 128], id128[:])
                        (nc.vector.tensor_copy if dt == 0 else nc.scalar.copy)(
                            QT[:, dt, g * 512:(g + 1) * 512], ps[:])
                    for et in range(ET):
                        ps = psA.tile([128, 512], F32, name="pq", tag="pa")
                        for dt in range(ET):
                            nc.tensor.matmul(
                                ps[:], w_r["Wq"][:, dt, et * 128:(et + 1) * 128],
                                QT[:, dt, g * 512:(g + 1) * 512],
                                start=(dt == 0), stop=(dt == ET - 1))
                        dst = qT[:, et, g * 512:(g + 1) * 512]
                        if use_bq:
                            nc.vector.tensor_scalar_add(dst, ps[:], vecs["bq"][:, et:et + 1])
                        else:
                            (nc.vector.tensor_copy if et == 0 else nc.scalar.copy)(
                                dst, ps[:])

            # ------------- phase B: attention (all blocks) -------------
            On = bpool.tile([128, 8, D], F32)
            with (
                tc.tile_pool(name="ut", bufs=3) as utp,
                tc.tile_pool(name="sm", bufs=2) as smp,
            ):
                with (
                    tc.tile_pool(name="scps", bufs=2, space="PSUM") as scps,
                    tc.tile_pool(name="accps", bufs=2, space="PSUM") as accps,
                ):
                    def attention_mm(hp, qb):
                        qsl = slice(qb * 512, (qb + 1) * 512)
                        acc = [accps.tile([65, 512], F32, name=f"acc{hp}{qb}{_h}", tag=f"acc{_h}")
                               for _h in range(2)]
                        uts = {}
                        # software-pipelined: scores/exp(kt) overlap A@V(kt-1)
                        for kt in range(KT + 1):
                            if kt < KT:
                                sc = scps.tile([128, 1024], F32, name="sc", tag="sc")
                                for hh in range(2):
                                    off = hh * 64
                                    nc.tensor.matmul(
                                        sc[:, hh * 512:(hh + 1) * 512],
                                        kT[off:off + 64, hp, kt * 128:(kt + 1) * 128],
                                        qT[off:off + 64, hp, qsl],
                                        start=True, stop=True)
                                ut = utp.tile([128, 1024], F32R)
                                nc.scalar.activation(ut[:], sc[:], AF.Exp, scale=SCALE)
                                uts[kt] = ut
                            if kt >= 1:
                                utp_ = uts.pop(kt - 1)
                                for hh in range(2):
                                    h = hp * 2 + hh
                                    nc.tensor.matmul(
                                        acc[hh][:],
                                        v_sb[:, kt - 1, h, :],
                                        utp_[:, hh * 512:(hh + 1) * 512],
                                        start=(kt - 1 == 0), stop=(kt - 1 == KT - 1))
                        return acc

                    def attention_norm(hp, qb, acc):
                        qsl = slice(qb * 512, (qb + 1) * 512)
                        for hh in range(2):
                            dcp = smp.tile([65, 512], F32, name=f"dcp{hh}", tag="dcp")
                            nc.vector.tensor_copy(dcp[64:65, :], acc[hh][64:65, :])
                            den0 = smp.tile([1, 512], F32, name=f"den0{hh}", tag="den0")
                            nc.sync.dma_start(den0[0:1, :], dcp[64:65, :])
                            rec = smp.tile([1, 512], F32, name=f"rec{hh}", tag="rec")
                            nc.vector.reciprocal_approx_fast(out=rec[:], in_=den0[:])
                            recBC = smp.tile([64, 512], F32, name=f"recBC{hh}", tag="recBC")
                            nc.gpsimd.partition_broadcast(recBC[:], rec[0:1, :])
                            tmp = smp.tile([64, 512], F32, name=f"tmp{hh}", tag="tmp")
                            nc.vector.tensor_mul(tmp[:], acc[hh][0:64, :], recBC[:])
                            if hh == 0:
                                nc.vector.tensor_add(OT[0:64, hp, qsl], tmp[:],
                                                     qT[0:64, hp, qsl])
                            else:
                                tsh = smp.tile([128, 512], F32, name="tsh", tag="tsh")
                                nc.sync.dma_start(tsh[64:128, :], tmp[:])
                                nc.vector.tensor_add(OT[64:128, hp, qsl], tsh[64:128, :],
                                                     qT[64:128, hp, qsl])
                        if use_bv:
                            nc.vector.tensor_scalar_add(OT[:, hp, qsl], OT[:, hp, qsl],
                                                        vecs["bv"][:, hp:hp + 1])

                    prev = None
                    for qb in range(QB):
                        for hp in range(2):
                            acc = attention_mm(hp, qb)
                            if prev is not None:
                                attention_norm(*prev)
                            prev = (hp, qb, acc)
                    attention_norm(*prev)

                # ------------- phase C: LN0 -> FFN -> LN1 -> store -------------
                with (
                    tc.tile_pool(name="lnps", bufs=2, space="PSUM") as lnps,
                    tc.tile_pool(name="ffps", bufs=2, space="PSUM") as ffps,
                    tc.tile_pool(name="ops", bufs=2, space="PSUM") as ops,
                    tc.tile_pool(name="lnsm", bufs=2) as lnsm,
                    tc.tile_pool(name="lnsq", bufs=2) as lnsq,
                    tc.tile_pool(name="ffsm", bufs=2) as ffsm,
                ):
                    def layernorm(x, y, qb, gname, bname, use_g):
                        qsl = slice(qb * 512, (qb + 1) * 512)
                        xsq = lnsq.tile([128, ET, 512], F32R)
                        for et in range(ET):
                            nc.gpsimd.tensor_mul(xsq[:, et, :], x[:, et, qsl], x[:, et, qsl])
                        mus = lnps.tile([1, 512], F32, name="mus", tag="mus")
                        sqs = lnps.tile([1, 512], F32, name="sqs", tag="sqs")
                        for et in range(ET):
                            nc.tensor.matmul(mus[:], onescr[:], x[:, et, qsl],
                                             start=(et == 0), stop=(et == ET - 1))
                            nc.tensor.matmul(sqs[:], onescr[:], xsq[:, et, :],
                                             start=(et == 0), stop=(et == ET - 1))
                        mu0 = lnsm.tile([1, 512], F32, name="mu0", tag="mu0")
                        nc.vector.tensor_copy(mu0[:], mus[:])
                        musq = lnsm.tile([1, 512], F32, name="musq", tag="musq")
                        nc.vector.tensor_mul(musq[:], mu0[:], mu0[:])
                        var = lnsm.tile([1, 512], F32, name="var", tag="var")
                        nc.vector.tensor_sub(var[:], sqs[:], musq[:])
                        lnv = lnsm.tile([1, 512], F32, name="lnv", tag="lnv")
                        nc.scalar.activation(lnv[:], var[:], AF.Ln, bias=epst[:])
                        rst = lnsm.tile([1, 512], F32, name="rst", tag="rst")
                        nc.scalar.activation(rst[:], lnv[:], AF.Exp, scale=-0.5)
                        muB = lnsm.tile([128, 512], F32, name="muB", tag="muB")
                        nc.gpsimd.partition_broadcast(muB[:], mu0[0:1, :])
                        rsB = lnsm.tile([128, 512], F32, name="rsB", tag="rsB")
                        nc.gpsimd.partition_broadcast(rsB[:], rst[0:1, :])
                        for et in range(ET):
                            cen = lnsm.tile([128, 512], F32, name="cen", tag="cen")
                            nc.vector.tensor_sub(cen[:], x[:, et, qsl], muB[:])
                            dst = y[:, et, qsl]
                            nc.vector.tensor_mul(dst, cen[:], rsB[:])
                            if use_g:
                                nc.vector.tensor_scalar(
                                    dst, dst, vecs[gname][:, et:et + 1],
                                    vecs[bname][:, et:et + 1], ALU.mult, ALU.add)

                    for qb in range(QB):
                        layernorm(OT, O1, qb, "g0", "b0", use_g0)
                    for qb in range(QB):
                        qsl = slice(qb * 512, (qb + 1) * 512)
                        for et in range(ET):
                            ps = ffps.tile([128, 512], F32)
                            for dt in range(ET):
                                nc.tensor.matmul(
                                    ps[:], w_r["Wo"][:, dt, et * 128:(et + 1) * 128],
                                    O1[:, dt, qsl],
                                    start=(dt == 0), stop=(dt == ET - 1))
                            ft = ffsm.tile([128, 512], F32, name="ft", tag="ft")
                            nc.vector.tensor_scalar(
                                ft[:], ps[:], vecs["bo"][:, et:et + 1] if use_bo else 0.0,
                                0.0, ALU.add, ALU.max)
                            nc.gpsimd.tensor_add(O2[:, et, qsl], O1[:, et, qsl], ft[:])
                    for qb in range(QB):
                        layernorm(O2, O3, qb, "g1", "b1", use_g1)
                    for qb in range(QB):
                        for st in range(4):
                            po = ops.tile([128, 256], F32)
                            for dt in range(ET):
                                nc.tensor.transpose(
                                    po[:, dt * 128:(dt + 1) * 128],
                                    O3[:, dt, qb * 512 + st * 128:qb * 512 + (st + 1) * 128],
                                    id128[:])
                            (nc.vector.tensor_copy if st % 2 == 0 else nc.scalar.copy)(
                                On[:, qb * 4 + st, :], po[:])
                        nc.sync.dma_start(
                            dO.rearrange("(g st p) d -> g p st d", p=128, st=4)[qb],
                            On[:, qb * 4:(qb + 1) * 4, :])

    nc.compile()
    return nc


def kernel(Q, K, Wq, bq, Wk, bk, Wv, bv, Wo, bo, g0, b0, g1, b1):
    Q, K = np.asarray(Q), np.asarray(K)
    ws = {n: np.ascontiguousarray(np.asarray(v), dtype=np.float32)
          for n, v in (("Wq", Wq), ("Wk", Wk), ("Wv", Wv), ("Wo", Wo))}
    vs = {n: np.ascontiguousarray(np.asarray(v), dtype=np.float32)
          for n, v in (("bq", bq), ("bk", bk), ("bv", bv), ("bo", bo),
                       ("g0", g0), ("b0", b0), ("g1", g1), ("b1", b1))}
    flags = (bool(np.any(vs["bq"])), bool(np.any(vs["bk"])),
             bool(np.any(vs["bv"])), bool(np.any(vs["bo"])),
             bool(np.any(vs["g0"] != 1.0) or np.any(vs["b0"])),
             bool(np.any(vs["g1"] != 1.0) or np.any(vs["b1"])))
    if flags not in _CACHE:
        _CACHE[flags] = _build(flags)
    nc = _CACHE[flags]

    in_maps = []
    for b in range(B):
        for half in range(2):
            m = {"Qs": np.ascontiguousarray(Q[b, half * S:(half + 1) * S], dtype=np.float32),
                 "Ks": np.ascontiguousarray(K[b], dtype=np.float32)}
            m.update(ws)
            m.update(vs)
            in_maps.append(m)

    res = run_bass_kernel_spmd(nc, in_maps, list(range(8)))
    out = np.empty((B, NQ, D), dtype=np.float32)
    for i in range(8):
        b, half = divmod(i, 2)
        out[b, half * S:(half + 1) * S] = res.results[i]["Out"]
    return out



# revision 18
# speedup vs baseline: 5.6396x; 1.0111x over previous
"""MAB (multihead attention block) TRN2 kernel.

Sharding: 8 cores = batch (4) x query-half (2). Each core computes its
[1024, 256] output slice with zero cross-core communication (K/V
projections are recomputed by the 2 cores sharing a batch).

Layout strategy: everything transposed (features on partitions) so that
- projections contract d on partitions,
- scores come out as [k, q] (exp output directly usable as A@V rhs),
- softmax denominators via a ones-row appended to V (PE does the sum),
- LN stats via ones-vector matmuls + K=1 broadcast matmuls (PE),
- FFN contracts e on partitions directly.
All matmuls run in float32r (~1.3e-4 rel err, full PE rate).
"""

import numpy as np

import concourse.bass as bass
import concourse.mybir as mybir
import concourse.tile as tile
from concourse import bacc
from concourse import masks
from concourse.bass_utils import run_bass_kernel_spmd

F32 = mybir.dt.float32
F32R = mybir.dt.float32r
AF = mybir.ActivationFunctionType
ALU = mybir.AluOpType

B, NQ, NK, D = 4, 2048, 2048, 256
H, DH = 4, 64
S = NQ // 2          # queries per core
ET = D // 128        # feature tiles
QB = S // 512        # query blocks of 512
KT = NK // 128       # key tiles of 128
KB = NK // 512       # key blocks of 512
EPS = 1e-5
SCALE = 1.0 / np.sqrt(D)

_CACHE = {}


def _build(flags):
    (use_bq, use_bk, use_bv, use_bo, use_g0, use_g1) = flags
    nc = bacc.Bacc(None, target_bir_lowering=False)

    dQ = nc.dram_tensor("Qs", [S, D], F32, kind="ExternalInput")
    dK = nc.dram_tensor("Ks", [NK, D], F32, kind="ExternalInput")
    dW = {w: nc.dram_tensor(w, [D, D], F32, kind="ExternalInput")
          for w in ("Wq", "Wk", "Wv", "Wo")}
    dV = {v: nc.dram_tensor(v, [D], F32, kind="ExternalInput")
          for v in ("bq", "bk", "bv", "bo", "g0", "b0", "g1", "b1")}
    dO = nc.dram_tensor("Out", [S, D], F32, kind="ExternalOutput")

    with tile.TileContext(nc) as tc:
        with (
            tc.tile_pool(name="const", bufs=1) as cpool,
            tc.tile_pool(name="acts", bufs=1) as apool,
            tc.tile_pool(name="big", bufs=1) as bpool,
        ):
            # ---------------- constants ----------------
            id128 = cpool.tile([128, 128], F32)
            masks.make_identity(nc, id128[:])
            w_r = {}
            for w in ("Wq", "Wk", "Wv", "Wo"):
                w_r[w] = cpool.tile([128, ET, D], F32R, name=f"wr_{w}", tag=f"wr_{w}")
            ones1 = cpool.tile([1, 128], F32)
            nc.vector.memset(ones1[:], 1.0)
            ones1r = cpool.tile([1, 128], F32R)
            nc.vector.tensor_copy(ones1r[:], ones1[:])
            onesc = cpool.tile([128, 1], F32)
            nc.vector.memset(onesc[:], 1.0 / D)
            onescr = cpool.tile([128, 1], F32R)
            nc.vector.tensor_copy(onescr[:], onesc[:])
            onesf0 = cpool.tile([128, 128], F32)
            nc.vector.memset(onesf0[:], 1.0)
            onesF = cpool.tile([128, 128], F32R)
            nc.vector.tensor_copy(onesF[:], onesf0[:])
            epst = cpool.tile([1, 1], F32)
            nc.vector.memset(epst[:], EPS)

            QT = apool.tile([128, ET, S], F32R)       # raw Q^T
            KTr = apool.tile([128, ET, NK], F32R)     # raw K^T
            qT = bpool.tile([128, ET, S], F32R)       # projected q, transposed
            kT = bpool.tile([128, ET, NK], F32R)      # projected k, transposed
            v_sb = bpool.tile([128, KT, H, 65], F32R)  # v [k, h, dh + ones col]
            OT = bpool.tile([128, ET, S], F32R)       # attention out + residual
            O1 = bpool.tile([128, ET, S], F32R)       # LN0 out
            O2 = bpool.tile([128, ET, S], F32R)       # FFN+residual out
            O3 = bpool.tile([128, ET, S], F32)        # LN1 out (transposed)
            nc.vector.tensor_copy(v_sb[:, :, :, 64:65], onesf0[:, 0:KT * H])

            # ---------- phase A: load natural, transpose on PE, project ----------
            # DMA order: weights (small, gate the projections), K chunks
            # (gate attention via kT/v_sb), Q chunks. Each K chunk is
            # transposed and projected while the next chunk transfers.
            with (
                tc.tile_pool(name="stage", bufs=1) as stpool,
                tc.tile_pool(name="tps", bufs=2, space="PSUM") as tps,
                tc.tile_pool(name="psA", bufs=4, space="PSUM") as psA,
            ):
                wn = {}
                for w in ("Wq", "Wk", "Wv", "Wo"):
                    wn[w] = stpool.tile([128, ET, D], F32, name=f"wn_{w}", tag=f"wn_{w}")
                    nc.sync.dma_start(
                        wn[w][:], dW[w].rearrange("(et p) d -> p et d", p=128))
                kn = stpool.tile([128, KT, D], F32)
                for g in range(4):
                    nc.sync.dma_start(
                        kn[:, g * 4:(g + 1) * 4, :],
                        dK.rearrange("(g st p) d -> g p st d", p=128, st=4)[g])
                qn = stpool.tile([128, 8, D], F32)
                for g in range(2):
                    nc.sync.dma_start(
                        qn[:, g * 4:(g + 1) * 4, :],
                        dQ.rearrange("(g st p) d -> g p st d", p=128, st=4)[g])
                vecs = {}
                need = {"bq": use_bq, "bk": use_bk, "bv": use_bv, "bo": use_bo,
                        "g0": use_g0, "b0": use_g0, "g1": use_g1, "b1": use_g1}
                for v in ("bq", "bk", "bv", "bo", "g0", "b0", "g1", "b1"):
                    if not need[v]:
                        continue
                    t = cpool.tile([128, ET], F32, name=f"vec_{v}", tag=f"vec_{v}")
                    nc.sync.dma_start(t[:], dV[v].rearrange("(et e) -> e et", e=128))
                    vecs[v] = t

                for w in ("Wq", "Wk", "Wv", "Wo"):
                    for dt in range(ET):
                        ps = tps.tile([128, 512], F32)
                        for et in range(ET):
                            nc.tensor.transpose(
                                ps[:, et * 128:(et + 1) * 128],
                                wn[w][:, et, dt * 128:(dt + 1) * 128], id128[:])
                        (nc.vector.tensor_copy if dt == 0 else nc.scalar.copy)(
                            w_r[w][:, dt, :], ps[:, 0:D])

                for g in range(4):          # K chunk: transpose + k/v-proj
                    for dt in range(ET):
                        ps = tps.tile([128, 512], F32)
                        for j in range(4):
                            nc.tensor.transpose(
                                ps[:, j * 128:(j + 1) * 128],
                                kn[:, g * 4 + j, dt * 128:(dt + 1) * 128], id128[:])
                        (nc.vector.tensor_copy if dt == 0 else nc.scalar.copy)(
                            KTr[:, dt, g * 512:(g + 1) * 512], ps[:])
                    for et in range(ET):
                        ps = psA.tile([128, 512], F32, name="pk", tag="pa")
                        for dt in range(ET):
                            nc.tensor.matmul(
                                ps[:], w_r["Wk"][:, dt, et * 128:(et + 1) * 128],
                                KTr[:, dt, g * 512:(g + 1) * 512],
                                start=(dt == 0), stop=(dt == ET - 1))
                        dst = kT[:, et, g * 512:(g + 1) * 512]
                        if use_bk:
                            nc.vector.tensor_scalar_add(dst, ps[:], vecs["bk"][:, et:et + 1])
                        else:
                            (nc.vector.tensor_copy if et == 0 else nc.scalar.copy)(
                                dst, ps[:])
                    for j in range(4):
                        kt = g * 4 + j
                        ps = psA.tile([128, 512], F32, name="pv", tag="pa")
                        for dt in range(ET):
                            nc.tensor.matmul(
                                ps[:, 0:256], KTr[:, dt, kt * 128:(kt + 1) * 128],
                                w_r["Wv"][:, dt, :],
                                start=(dt == 0), stop=(dt == ET - 1))
                        (nc.vector.tensor_copy if j % 2 == 0 else nc.scalar.copy)(
                            v_sb[:, kt, :, 0:64], ps[:, 0:256])

                for g in range(2):          # Q chunk: transpose + q-proj
                    for dt in range(ET):
                        ps = tps.tile([128, 512], F32)
                        for j in range(4):
                            nc.tensor.transpose(
                                ps[:, j * 128:(j + 1) * 128],
                                qn[:, g * 4 + j, dt * 128:(dt + 1) * 128], id128[:])
                        (nc.vector.tensor_copy if dt == 0 else nc.scalar.copy)(
                            QT[:, dt, g * 512:(g + 1) * 512], ps[:])
                    for et in range(ET):
                        ps = psA.tile([128, 512], F32, name="pq", tag="pa")
                        for dt in range(ET):
                            nc.tensor.matmul(
                                ps[:], w_r["Wq"][:, dt, et * 128:(et + 1) * 128],
                                QT[:, dt, g * 512:(g + 1) * 512],
                                start=(dt == 0), stop=(dt == ET - 1))
                        dst = qT[:, et, g * 512:(g + 1) * 512]
                        if use_bq:
                            nc.vector.tensor_scalar_add(dst, ps[:], vecs["bq"][:, et:et + 1])
                        else:
                            (nc.vector.tensor_copy if et == 0 else nc.scalar.copy)(
                                dst, ps[:])

            # ------------- phase B: attention (all blocks) -------------
            On = bpool.tile([128, 8, D], F32)
            with (
                tc.tile_pool(name="ut", bufs=3) as utp,
                tc.tile_pool(name="sm", bufs=2) as smp,
            ):
                with (
                    tc.tile_pool(name="scps", bufs=2, space="PSUM") as scps,
                    tc.tile_pool(name="accps", bufs=2, space="PSUM") as accps,
                ):
                    def attention_mm(hp, qb):
                        qsl = slice(qb * 512, (qb + 1) * 512)
                        acc = [accps.tile([65, 512], F32, name=f"acc{hp}{qb}{_h}", tag=f"acc{_h}")
                               for _h in range(2)]
                        uts = {}
                        # software-pipelined: scores/exp(kt) overlap A@V(kt-1)
                        for kt in range(KT + 1):
                            if kt < KT:
                                sc = scps.tile([128, 1024], F32, name="sc", tag="sc")
                                for hh in range(2):
                                    off = hh * 64
                                    nc.tensor.matmul(
                                        sc[:, hh * 512:(hh + 1) * 512],
                                        kT[off:off + 64, hp, kt * 128:(kt + 1) * 128],
                                        qT[off:off + 64, hp, qsl],
                                        start=True, stop=True)
                                ut = utp.tile([128, 1024], F32R)
                                nc.scalar.activation(ut[:], sc[:], AF.Exp, scale=SCALE)
                                uts[kt] = ut
                            if kt >= 1:
                                utp_ = uts.pop(kt - 1)
                                for hh in range(2):
                                    h = hp * 2 + hh
                                    nc.tensor.matmul(
                                        acc[hh][:],
                                        v_sb[:, kt - 1, h, :],
                                        utp_[:, hh * 512:(hh + 1) * 512],
                                        start=(kt - 1 == 0), stop=(kt - 1 == KT - 1))
                        return acc

                    def attention_norm(hp, qb, acc):
                        qsl = slice(qb * 512, (qb + 1) * 512)
                        for hh in range(2):
                            dcp = smp.tile([65, 512], F32, name=f"dcp{hh}", tag="dcp")
                            nc.vector.tensor_copy(dcp[64:65, :], acc[hh][64:65, :])
                            den0 = smp.tile([1, 512], F32, name=f"den0{hh}", tag="den0")
                            nc.sync.dma_start(den0[0:1, :], dcp[64:65, :])
                            rec = smp.tile([1, 512], F32, name=f"rec{hh}", tag="rec")
                            nc.vector.reciprocal_approx_fast(out=rec[:], in_=den0[:])
                            recBC = smp.tile([64, 512], F32, name=f"recBC{hh}", tag="recBC")
                            nc.gpsimd.partition_broadcast(recBC[:], rec[0:1, :])
                            tmp = smp.tile([64, 512], F32, name=f"tmp{hh}", tag="tmp")
                            nc.vector.tensor_mul(tmp[:], acc[hh][0:64, :], recBC[:])
                            if hh == 0:
                                nc.vector.tensor_add(OT[0:64, hp, qsl], tmp[:],
                                                     qT[0:64, hp, qsl])
                            else:
                                tsh = smp.tile([128, 512], F32, name="tsh", tag="tsh")
                                nc.sync.dma_start(tsh[64:128, :], tmp[:])
                                nc.vector.tensor_add(OT[64:128, hp, qsl], tsh[64:128, :],
                                                     qT[64:128, hp, qsl])
                        if use_bv:
                            nc.vector.tensor_scalar_add(OT[:, hp, qsl], OT[:, hp, qsl],
                                                        vecs["bv"][:, hp:hp + 1])

                    prev = None
                    for qb in range(QB):
                        for hp in range(2):
                            acc = attention_mm(hp, qb)
                            if prev is not None:
                                attention_norm(*prev)
                            prev = (hp, qb, acc)
                    attention_norm(*prev)

                # ------------- phase C: LN0 -> FFN -> LN1 -> store -------------
                with (
                    tc.tile_pool(name="lnps", bufs=2, space="PSUM") as lnps,
                    tc.tile_pool(name="ffps", bufs=2, space="PSUM") as ffps,
                    tc.tile_pool(name="ops", bufs=2, space="PSUM") as ops,
                    tc.tile_pool(name="lnsm", bufs=2) as lnsm,
                    tc.tile_pool(name="lnsq", bufs=2) as lnsq,
                    tc.tile_pool(name="ffsm", bufs=2) as ffsm,
                ):
                    def layernorm(x, y, qb, gname, bname, use_g):
                        qsl = slice(qb * 512, (qb + 1) * 512)
                        xsq = lnsq.tile([128, ET, 512], F32R)
                        for et in range(ET):
                            nc.scalar.activation(xsq[:, et, :], x[:, et, qsl], AF.Square)
                        mus = lnps.tile([1, 512], F32, name="mus", tag="mus")
                        sqs = lnps.tile([1, 512], F32, name="sqs", tag="sqs")
                        for et in range(ET):
                            nc.tensor.matmul(mus[:], onescr[:], x[:, et, qsl],
                                             start=(et == 0), stop=(et == ET - 1))
                            nc.tensor.matmul(sqs[:], onescr[:], xsq[:, et, :],
                                             start=(et == 0), stop=(et == ET - 1))
                        mu0 = lnsm.tile([1, 512], F32, name="mu0", tag="mu0")
                        nc.vector.tensor_copy(mu0[:], mus[:])
                        musq = lnsm.tile([1, 512], F32, name="musq", tag="musq")
                        nc.vector.tensor_mul(musq[:], mu0[:], mu0[:])
                        var = lnsm.tile([1, 512], F32, name="var", tag="var")
                        nc.vector.tensor_sub(var[:], sqs[:], musq[:])
                        lnv = lnsm.tile([1, 512], F32, name="lnv", tag="lnv")
                        nc.scalar.activation(lnv[:], var[:], AF.Ln, bias=epst[:])
                        rst = lnsm.tile([1, 512], F32, name="rst", tag="rst")
                        nc.scalar.activation(rst[:], lnv[:], AF.Exp, scale=-0.5)
                        muB = lnsm.tile([128, 512], F32, name="muB", tag="muB")
                        nc.gpsimd.partition_broadcast(muB[:], mu0[0:1, :])
                        rsB = lnsm.tile([128, 512], F32, name="rsB", tag="rsB")
                        nc.gpsimd.partition_broadcast(rsB[:], rst[0:1, :])
                        for et in range(ET):
                            cen = lnsm.tile([128, 512], F32, name="cen", tag="cen")
                            nc.vector.tensor_sub(cen[:], x[:, et, qsl], muB[:])
                            dst = y[:, et, qsl]
                            nc.vector.tensor_mul(dst, cen[:], rsB[:])
                            if use_g:
                                nc.vector.tensor_scalar(
                                    dst, dst, vecs[gname][:, et:et + 1],
                                    vecs[bname][:, et:et + 1], ALU.mult, ALU.add)

                    for qb in range(QB):
                        layernorm(OT, O1, qb, "g0", "b0", use_g0)
                    for qb in range(QB):
                        qsl = slice(qb * 512, (qb + 1) * 512)
                        for et in range(ET):
                            ps = ffps.tile([128, 512], F32)
                            for dt in range(ET):
                                nc.tensor.matmul(
                                    ps[:], w_r["Wo"][:, dt, et * 128:(et + 1) * 128],
                                    O1[:, dt, qsl],
                                    start=(dt == 0), stop=(dt == ET - 1))
                            if use_bo:
                                ft = ffsm.tile([128, 512], F32, name="ft", tag="ft")
                                nc.vector.tensor_scalar(
                                    ft[:], ps[:], vecs["bo"][:, et:et + 1],
                                    0.0, ALU.add, ALU.max)
                                nc.gpsimd.tensor_add(O2[:, et, qsl], O1[:, et, qsl], ft[:])
                            else:
                                nc.vector.scalar_tensor_tensor(
                                    O2[:, et, qsl], ps[:], 0.0, O1[:, et, qsl],
                                    ALU.max, ALU.add)
                    for qb in range(QB):
                        layernorm(O2, O3, qb, "g1", "b1", use_g1)
                    for qb in range(QB):
                        for st in range(4):
                            po = ops.tile([128, 256], F32)
                            for dt in range(ET):
                                nc.tensor.transpose(
                                    po[:, dt * 128:(dt + 1) * 128],
                                    O3[:, dt, qb * 512 + st * 128:qb * 512 + (st + 1) * 128],
                                    id128[:])
                            (nc.vector.tensor_copy if st % 2 == 0 else nc.scalar.copy)(
                                On[:, qb * 4 + st, :], po[:])
                        nc.sync.dma_start(
                            dO.rearrange("(g st p) d -> g p st d", p=128, st=4)[qb],
                            On[:, qb * 4:(qb + 1) * 4, :])

    nc.compile()
    return nc


def kernel(Q, K, Wq, bq, Wk, bk, Wv, bv, Wo, bo, g0, b0, g1, b1):
    Q, K = np.asarray(Q), np.asarray(K)
    ws = {n: np.ascontiguousarray(np.asarray(v), dtype=np.float32)
          for n, v in (("Wq", Wq), ("Wk", Wk), ("Wv", Wv), ("Wo", Wo))}
    vs = {n: np.ascontiguousarray(np.asarray(v), dtype=np.float32)
          for n, v in (("bq", bq), ("bk", bk), ("bv", bv), ("bo", bo),
                       ("g0", g0), ("b0", b0), ("g1", g1), ("b1", b1))}
    flags = (bool(np.any(vs["bq"])), bool(np.any(vs["bk"])),
             bool(np.any(vs["bv"])), bool(np.any(vs["bo"])),
             bool(np.any(vs["g0"] != 1.0) or np.any(vs["b0"])),
             bool(np.any(vs["g1"] != 1.0) or np.any(vs["b1"])))
    if flags not in _CACHE:
        _CACHE[flags] = _build(flags)
    nc = _CACHE[flags]

    in_maps = []
    for b in range(B):
        for half in range(2):
            m = {"Qs": np.ascontiguousarray(Q[b, half * S:(half + 1) * S], dtype=np.float32),
                 "Ks": np.ascontiguousarray(K[b], dtype=np.float32)}
            m.update(ws)
            m.update(vs)
            in_maps.append(m)

    res = run_bass_kernel_spmd(nc, in_maps, list(range(8)))
    out = np.empty((B, NQ, D), dtype=np.float32)
    for i in range(8):
        b, half = divmod(i, 2)
        out[b, half * S:(half + 1) * S] = res.results[i]["Out"]
    return out



# revision 20
# speedup vs baseline: 5.8620x; 1.0394x over previous
"""MAB (multihead attention block) TRN2 kernel.

Sharding: 8 cores = batch (4) x query-half (2). Each core computes its
[1024, 256] output slice with zero cross-core communication (K/V
projections are recomputed by the 2 cores sharing a batch).

Layout strategy: everything transposed (features on partitions) so that
- projections contract d on partitions,
- scores come out as [k, q] (exp output directly usable as A@V rhs),
- softmax denominators via a ones-row appended to V (PE does the sum),
- LN stats via ones-vector matmuls + K=1 broadcast matmuls (PE),
- FFN contracts e on partitions directly.
All matmuls run in float32r (~1.3e-4 rel err, full PE rate).
"""

import numpy as np

import concourse.bass as bass
import concourse.mybir as mybir
import concourse.tile as tile
from concourse import bacc
from concourse import masks
from concourse.bass_utils import run_bass_kernel_spmd

F32 = mybir.dt.float32
F32R = mybir.dt.float32r
AF = mybir.ActivationFunctionType
ALU = mybir.AluOpType

B, NQ, NK, D = 4, 2048, 2048, 256
H, DH = 4, 64
S = NQ // 2          # queries per core
ET = D // 128        # feature tiles
QB = S // 512        # query blocks of 512
KT = NK // 128       # key tiles of 128
KB = NK // 512       # key blocks of 512
EPS = 1e-5
SCALE = 1.0 / np.sqrt(D)

_CACHE = {}


def _build(flags):
    (use_bq, use_bk, use_bv, use_bo, use_g0, use_g1) = flags
    nc = bacc.Bacc(None, target_bir_lowering=False)

    dQ = nc.dram_tensor("Qs", [S, D], F32, kind="ExternalInput")
    dK = nc.dram_tensor("Ks", [NK, D], F32, kind="ExternalInput")
    dW = {w: nc.dram_tensor(w, [D, D], F32, kind="ExternalInput")
          for w in ("Wq", "Wk", "Wv", "Wo")}
    dV = {v: nc.dram_tensor(v, [D], F32, kind="ExternalInput")
          for v in ("bq", "bk", "bv", "bo", "g0", "b0", "g1", "b1")}
    dO = nc.dram_tensor("Out", [S, D], F32, kind="ExternalOutput")

    with tile.TileContext(nc) as tc:
        with (
            tc.tile_pool(name="const", bufs=1) as cpool,
            tc.tile_pool(name="acts", bufs=1) as apool,
            tc.tile_pool(name="big", bufs=1) as bpool,
        ):
            # Pin the ACT LUT set that covers Exp+Ln+Square+Copy so the
            # table-load pass never needs to switch tables mid-kernel.
            from concourse.bacc import get_activation_tables
            _tabs = list(get_activation_tables(nc.m.arch))
            nc.scalar.add_instruction(mybir.InstLoadActFuncSet(
                name=nc.get_next_instruction_name(),
                act_func_set_id=_tabs.index("natural_log_exp_and_others"),
                ins=[], outs=[]))
            # ---------------- constants ----------------
            onesf0 = cpool.tile([128, 128], F32)
            nc.vector.memset(onesf0[:], 1.0)
            # PE warmup: dummy transposes ramp the tensor engine to full
            # clock while the input DMAs are still in flight.
            with tc.tile_pool(name="warm", bufs=2, space="PSUM") as wps:
                for _i in range(8):
                    pw = wps.tile([128, 128], F32, name="pw", tag="pw")
                    nc.tensor.transpose(pw[:], onesf0[:], onesf0[:])
            id128 = cpool.tile([128, 128], F32)
            masks.make_identity(nc, id128[:])
            w_r = {}
            for w in ("Wq", "Wk", "Wv", "Wo"):
                w_r[w] = cpool.tile([128, ET, D], F32R, name=f"wr_{w}", tag=f"wr_{w}")
            ones1 = cpool.tile([1, 128], F32)
            nc.vector.memset(ones1[:], 1.0)
            ones1r = cpool.tile([1, 128], F32R)
            nc.vector.tensor_copy(ones1r[:], ones1[:])
            onesc = cpool.tile([128, 1], F32)
            nc.vector.memset(onesc[:], 1.0 / D)
            onescr = cpool.tile([128, 1], F32R)
            nc.vector.tensor_copy(onescr[:], onesc[:])
            onesF = cpool.tile([128, 128], F32R)
            nc.vector.tensor_copy(onesF[:], onesf0[:])
            epst = cpool.tile([1, 1], F32)
            nc.vector.memset(epst[:], EPS)

            QT = apool.tile([128, ET, S], F32R)       # raw Q^T
            KTr = apool.tile([128, ET, NK], F32R)     # raw K^T
            qT = bpool.tile([128, ET, S], F32R)       # projected q, transposed
            kT = bpool.tile([128, ET, NK], F32R)      # projected k, transposed
            v_sb = bpool.tile([128, KT, H, 65], F32R)  # v [k, h, dh + ones col]
            OT = bpool.tile([128, ET, S], F32R)       # attention out + residual
            O1 = bpool.tile([128, ET, S], F32R)       # LN0 out
            O2 = bpool.tile([128, ET, S], F32R)       # FFN+residual out
            O3 = bpool.tile([128, ET, S], F32)        # LN1 out (transposed)
            nc.vector.tensor_copy(v_sb[:, :, :, 64:65], onesf0[:, 0:KT * H])

            # ---------- phase A: load natural, transpose on PE, project ----------
            # DMA order: weights (small, gate the projections), K chunks
            # (gate attention via kT/v_sb), Q chunks. Each K chunk is
            # transposed and projected while the next chunk transfers.
            with (
                tc.tile_pool(name="stage", bufs=1) as stpool,
                tc.tile_pool(name="tps", bufs=2, space="PSUM") as tps,
                tc.tile_pool(name="psA", bufs=4, space="PSUM") as psA,
            ):
                wn = {}
                for w in ("Wq", "Wk", "Wv", "Wo"):
                    wn[w] = stpool.tile([128, ET, D], F32, name=f"wn_{w}", tag=f"wn_{w}")
                    nc.sync.dma_start(
                        wn[w][:], dW[w].rearrange("(et p) d -> p et d", p=128))
                kn = stpool.tile([128, KT, D], F32)
                for g in range(4):
                    nc.sync.dma_start(
                        kn[:, g * 4:(g + 1) * 4, :],
                        dK.rearrange("(g st p) d -> g p st d", p=128, st=4)[g])
                qn = stpool.tile([128, 8, D], F32)
                for g in range(2):
                    nc.sync.dma_start(
                        qn[:, g * 4:(g + 1) * 4, :],
                        dQ.rearrange("(g st p) d -> g p st d", p=128, st=4)[g])
                vecs = {}
                need = {"bq": use_bq, "bk": use_bk, "bv": use_bv, "bo": use_bo,
                        "g0": use_g0, "b0": use_g0, "g1": use_g1, "b1": use_g1}
                for v in ("bq", "bk", "bv", "bo", "g0", "b0", "g1", "b1"):
                    if not need[v]:
                        continue
                    t = cpool.tile([128, ET], F32, name=f"vec_{v}", tag=f"vec_{v}")
                    nc.sync.dma_start(t[:], dV[v].rearrange("(et e) -> e et", e=128))
                    vecs[v] = t

                for w in ("Wq", "Wk", "Wv", "Wo"):
                    for dt in range(ET):
                        ps = tps.tile([128, 512], F32)
                        for et in range(ET):
                            nc.tensor.transpose(
                                ps[:, et * 128:(et + 1) * 128],
                                wn[w][:, et, dt * 128:(dt + 1) * 128], id128[:])
                        (nc.vector.tensor_copy if dt == 0 else nc.scalar.copy)(
                            w_r[w][:, dt, :], ps[:, 0:D])

                for g in range(4):          # K chunk: transpose + k/v-proj
                    for dt in range(ET):
                        ps = tps.tile([128, 512], F32)
                        for j in range(4):
                            nc.tensor.transpose(
                                ps[:, j * 128:(j + 1) * 128],
                                kn[:, g * 4 + j, dt * 128:(dt + 1) * 128], id128[:])
                        (nc.vector.tensor_copy if dt == 0 else nc.scalar.copy)(
                            KTr[:, dt, g * 512:(g + 1) * 512], ps[:])
                    for et in range(ET):
                        ps = psA.tile([128, 512], F32, name="pk", tag="pa")
                        for dt in range(ET):
                            nc.tensor.matmul(
                                ps[:], w_r["Wk"][:, dt, et * 128:(et + 1) * 128],
                                KTr[:, dt, g * 512:(g + 1) * 512],
                                start=(dt == 0), stop=(dt == ET - 1))
                        dst = kT[:, et, g * 512:(g + 1) * 512]
                        if use_bk:
                            nc.vector.tensor_scalar_add(dst, ps[:], vecs["bk"][:, et:et + 1])
                        else:
                            (nc.vector.tensor_copy if et == 0 else nc.scalar.copy)(
                                dst, ps[:])
                    for j in range(4):
                        kt = g * 4 + j
                        ps = psA.tile([128, 512], F32, name="pv", tag="pa")
                        for dt in range(ET):
                            nc.tensor.matmul(
                                ps[:, 0:256], KTr[:, dt, kt * 128:(kt + 1) * 128],
                                w_r["Wv"][:, dt, :],
                                start=(dt == 0), stop=(dt == ET - 1))
                        (nc.vector.tensor_copy if j % 2 == 0 else nc.scalar.copy)(
                            v_sb[:, kt, :, 0:64], ps[:, 0:256])

                for g in range(2):          # Q chunk: transpose + q-proj
                    for dt in range(ET):
                        ps = tps.tile([128, 512], F32)
                        for j in range(4):
                            nc.tensor.transpose(
                                ps[:, j * 128:(j + 1) * 128],
                                qn[:, g * 4 + j, dt * 128:(dt + 1) * 128], id128[:])
                        (nc.vector.tensor_copy if dt == 0 else nc.scalar.copy)(
                            QT[:, dt, g * 512:(g + 1) * 512], ps[:])
                    for et in range(ET):
                        ps = psA.tile([128, 512], F32, name="pq", tag="pa")
                        for dt in range(ET):
                            nc.tensor.matmul(
                                ps[:], w_r["Wq"][:, dt, et * 128:(et + 1) * 128],
                                QT[:, dt, g * 512:(g + 1) * 512],
                                start=(dt == 0), stop=(dt == ET - 1))
                        dst = qT[:, et, g * 512:(g + 1) * 512]
                        if use_bq:
                            nc.vector.tensor_scalar_add(dst, ps[:], vecs["bq"][:, et:et + 1])
                        else:
                            (nc.vector.tensor_copy if et == 0 else nc.scalar.copy)(
                                dst, ps[:])

            # ------------- phase B: attention (all blocks) -------------
            On = bpool.tile([128, 8, D], F32)
            with (
                tc.tile_pool(name="ut", bufs=3) as utp,
                tc.tile_pool(name="sm", bufs=2) as smp,
            ):
                with (
                    tc.tile_pool(name="scps", bufs=2, space="PSUM") as scps,
                    tc.tile_pool(name="accps", bufs=2, space="PSUM") as accps,
                ):
                    def attention_mm(hp, qb):
                        qsl = slice(qb * 512, (qb + 1) * 512)
                        acc = [accps.tile([65, 512], F32, name=f"acc{hp}{qb}{_h}", tag=f"acc{_h}")
                               for _h in range(2)]
                        uts = {}
                        # software-pipelined: scores/exp(kt) overlap A@V(kt-1)
                        for kt in range(KT + 1):
                            if kt < KT:
                                sc = scps.tile([128, 1024], F32, name="sc", tag="sc")
                                for hh in range(2):
                                    off = hh * 64
                                    nc.tensor.matmul(
                                        sc[:, hh * 512:(hh + 1) * 512],
                                        kT[off:off + 64, hp, kt * 128:(kt + 1) * 128],
                                        qT[off:off + 64, hp, qsl],
                                        start=True, stop=True)
                                ut = utp.tile([128, 1024], F32R)
                                nc.scalar.activation(ut[:], sc[:], AF.Exp, scale=SCALE)
                                uts[kt] = ut
                            if kt >= 1:
                                utp_ = uts.pop(kt - 1)
                                for hh in range(2):
                                    h = hp * 2 + hh
                                    nc.tensor.matmul(
                                        acc[hh][:],
                                        v_sb[:, kt - 1, h, :],
                                        utp_[:, hh * 512:(hh + 1) * 512],
                                        start=(kt - 1 == 0), stop=(kt - 1 == KT - 1))
                        return acc

                    def attention_norm(hp, qb, acc):
                        qsl = slice(qb * 512, (qb + 1) * 512)
                        for hh in range(2):
                            dcp = smp.tile([65, 512], F32, name=f"dcp{hh}", tag="dcp")
                            nc.vector.tensor_copy(dcp[64:65, :], acc[hh][64:65, :])
                            den0 = smp.tile([1, 512], F32, name=f"den0{hh}", tag="den0")
                            nc.sync.dma_start(den0[0:1, :], dcp[64:65, :])
                            rec = smp.tile([1, 512], F32, name=f"rec{hh}", tag="rec")
                            nc.vector.reciprocal_approx_fast(out=rec[:], in_=den0[:])
                            recBC = smp.tile([64, 512], F32, name=f"recBC{hh}", tag="recBC")
                            nc.gpsimd.partition_broadcast(recBC[:], rec[0:1, :])
                            tmp = smp.tile([64, 512], F32, name=f"tmp{hh}", tag="tmp")
                            nc.vector.tensor_mul(tmp[:], acc[hh][0:64, :], recBC[:])
                            if hh == 0:
                                nc.vector.tensor_add(OT[0:64, hp, qsl], tmp[:],
                                                     qT[0:64, hp, qsl])
                            else:
                                tsh = smp.tile([128, 512], F32, name="tsh", tag="tsh")
                                nc.sync.dma_start(tsh[64:128, :], tmp[:])
                                nc.vector.tensor_add(OT[64:128, hp, qsl], tsh[64:128, :],
                                                     qT[64:128, hp, qsl])
                        if use_bv:
                            nc.vector.tensor_scalar_add(OT[:, hp, qsl], OT[:, hp, qsl],
                                                        vecs["bv"][:, hp:hp + 1])

                    prev = None
                    for qb in range(QB):
                        for hp in range(2):
                            acc = attention_mm(hp, qb)
                            if prev is not None:
                                attention_norm(*prev)
                            prev = (hp, qb, acc)
                    attention_norm(*prev)

                # ------------- phase C: LN0 -> FFN -> LN1 -> store -------------
                with (
                    tc.tile_pool(name="lnps", bufs=2, space="PSUM") as lnps,
                    tc.tile_pool(name="ffps", bufs=2, space="PSUM") as ffps,
                    tc.tile_pool(name="ops", bufs=2, space="PSUM") as ops,
                    tc.tile_pool(name="lnsm", bufs=2) as lnsm,
                    tc.tile_pool(name="lnsq", bufs=2) as lnsq,
                    tc.tile_pool(name="ffsm", bufs=2) as ffsm,
                ):
                    def layernorm(x, y, qb, gname, bname, use_g):
                        qsl = slice(qb * 512, (qb + 1) * 512)
                        xsq = lnsq.tile([128, ET, 512], F32R)
                        for et in range(ET):
                            nc.scalar.activation(xsq[:, et, :], x[:, et, qsl], AF.Square)
                        mus = lnps.tile([1, 512], F32, name="mus", tag="mus")
                        sqs = lnps.tile([1, 512], F32, name="sqs", tag="sqs")
                        for et in range(ET):
                            nc.tensor.matmul(mus[:], onescr[:], x[:, et, qsl],
                                             start=(et == 0), stop=(et == ET - 1))
                            nc.tensor.matmul(sqs[:], onescr[:], xsq[:, et, :],
                                             start=(et == 0), stop=(et == ET - 1))
                        mu0 = lnsm.tile([1, 512], F32, name="mu0", tag="mu0")
                        nc.scalar.copy(mu0[:], mus[:])
                        musq = lnsm.tile([1, 512], F32, name="musq", tag="musq")
                        nc.scalar.activation(musq[:], mus[:], AF.Square)
                        var = lnsm.tile([1, 512], F32, name="var", tag="var")
                        nc.vector.tensor_sub(var[:], sqs[:], musq[:])
                        lnv = lnsm.tile([1, 512], F32, name="lnv", tag="lnv")
                        nc.scalar.activation(lnv[:], var[:], AF.Ln, bias=epst[:])
                        rst = lnsm.tile([1, 512], F32, name="rst", tag="rst")
                        nc.scalar.activation(rst[:], lnv[:], AF.Exp, scale=-0.5)
                        muB = lnsm.tile([128, 512], F32, name="muB", tag="muB")
                        nc.gpsimd.partition_broadcast(muB[:], mu0[0:1, :])
                        rsB = lnsm.tile([128, 512], F32, name="rsB", tag="rsB")
                        nc.gpsimd.partition_broadcast(rsB[:], rst[0:1, :])
                        for et in range(ET):
                            eng = nc.vector if et == 0 else nc.gpsimd
                            cen = lnsm.tile([128, 512], F32, name="cen", tag="cen")
                            eng.tensor_sub(cen[:], x[:, et, qsl], muB[:])
                            dst = y[:, et, qsl]
                            eng.tensor_mul(dst, cen[:], rsB[:])
                            if use_g:
                                nc.vector.tensor_scalar(
                                    dst, dst, vecs[gname][:, et:et + 1],
                                    vecs[bname][:, et:et + 1], ALU.mult, ALU.add)

                    for qb in range(QB):
                        layernorm(OT, O1, qb, "g0", "b0", use_g0)
                    for qb in range(QB):
                        qsl = slice(qb * 512, (qb + 1) * 512)
                        for et in range(ET):
                            ps = ffps.tile([128, 512], F32)
                            for dt in range(ET):
                                nc.tensor.matmul(
                                    ps[:], w_r["Wo"][:, dt, et * 128:(et + 1) * 128],
                                    O1[:, dt, qsl],
                                    start=(dt == 0), stop=(dt == ET - 1))
                            if use_bo:
                                ft = ffsm.tile([128, 512], F32, name="ft", tag="ft")
                                nc.vector.tensor_scalar(
                                    ft[:], ps[:], vecs["bo"][:, et:et + 1],
                                    0.0, ALU.add, ALU.max)
                                nc.gpsimd.tensor_add(O2[:, et, qsl], O1[:, et, qsl], ft[:])
                            else:
                                nc.vector.scalar_tensor_tensor(
                                    O2[:, et, qsl], ps[:], 0.0, O1[:, et, qsl],
                                    ALU.max, ALU.add)
                    for qb in range(QB):
                        layernorm(O2, O3, qb, "g1", "b1", use_g1)
                    for qb in range(QB):
                        for st in range(4):
                            po = ops.tile([128, 256], F32)
                            for dt in range(ET):
                                nc.tensor.transpose(
                                    po[:, dt * 128:(dt + 1) * 128],
                                    O3[:, dt, qb * 512 + st * 128:qb * 512 + (st + 1) * 128],
                                    id128[:])
                            (nc.vector.tensor_copy if st % 2 == 0 else nc.scalar.copy)(
                                On[:, qb * 4 + st, :], po[:])
                        nc.sync.dma_start(
                            dO.rearrange("(g st p) d -> g p st d", p=128, st=4)[qb],
                            On[:, qb * 4:(qb + 1) * 4, :])

    nc.compile()
    return nc


def kernel(Q, K, Wq, bq, Wk, bk, Wv, bv, Wo, bo, g0, b0, g1, b1):
    Q, K = np.asarray(Q), np.asarray(K)
    ws = {n: np.ascontiguousarray(np.asarray(v), dtype=np.float32)
          for n, v in (("Wq", Wq), ("Wk", Wk), ("Wv", Wv), ("Wo", Wo))}
    vs = {n: np.ascontiguousarray(np.asarray(v), dtype=np.float32)
          for n, v in (("bq", bq), ("bk", bk), ("bv", bv), ("bo", bo),
                       ("g0", g0), ("b0", b0), ("g1", g1), ("b1", b1))}
    flags = (bool(np.any(vs["bq"])), bool(np.any(vs["bk"])),
             bool(np.any(vs["bv"])), bool(np.any(vs["bo"])),
             bool(np.any(vs["g0"] != 1.0) or np.any(vs["b0"])),
             bool(np.any(vs["g1"] != 1.0) or np.any(vs["b1"])))
    if flags not in _CACHE:
        _CACHE[flags] = _build(flags)
    nc = _CACHE[flags]

    in_maps = []
    for b in range(B):
        for half in range(2):
            m = {"Qs": np.ascontiguousarray(Q[b, half * S:(half + 1) * S], dtype=np.float32),
                 "Ks": np.ascontiguousarray(K[b], dtype=np.float32)}
            m.update(ws)
            m.update(vs)
            in_maps.append(m)

    res = run_bass_kernel_spmd(nc, in_maps, list(range(8)))
    out = np.empty((B, NQ, D), dtype=np.float32)
    for i in range(8):
        b, half = divmod(i, 2)
        out[b, half * S:(half + 1) * S] = res.results[i]["Out"]
    return out

